# revision 1
# baseline (speedup 1.0000x reference)
# DeepseekV3MoECalibrate Trainium2 kernel (8 NeuronCores, expert-parallel).
#
# Sharding: 32 experts -> 4 per core; shared expert split along the 2I=2048
# intermediate dim (256 per core, processed as one 2-i-tile pseudo-expert);
# tokens replicated; partial outputs summed with an on-device AllReduce.
#
# Numerics: router (sigmoid top-k selection) in fp32; expert/shared MLP
# matmuls in fp32r (full PE rate at N=512, ~1.6e-4 rel err); combine weights
# applied before the down-projection so each entry's contribution is a plain
# sum accumulated in SBUF.
#
# Schedule: weight transposition for entry k+1 is software-pipelined into
# entry k's down-projection (stage-3) matmul stream so the PE never crawls
# through an evacuation-gated transpose phase.
from contextlib import ExitStack

import numpy as np

import concourse.bass as bass
import concourse.tile as tile
from concourse import bacc, mybir
from concourse.masks import make_identity

F32 = mybir.dt.float32
F32R = mybir.dt.float32r
AF = mybir.ActivationFunctionType
OP = mybir.AluOpType
AX = mybir.AxisListType

N_CORES = 8
T, H, I, E = 1024, 1024, 512, 32
E_LOC = E // N_CORES          # 4 experts per core
ISH = 2 * I // N_CORES        # 256 shared-intermediate rows per core
TT = T // 128                 # 8 token tiles
HK = H // 128                 # 8 h k-tiles
IK = I // 128                 # 4 i-tiles per expert
SK = ISH // 128               # 2 shared i-tiles
NH = H // 512                 # 2 h halves (stage-3 rhs)
TH = T // 512                 # 2 t halves (stage-1 rhs)


def build_module(use_collective=True, num_devices=N_CORES):
    nc = bacc.Bacc("TRN2", target_bir_lowering=False, debug=False,
                   num_devices=num_devices)

    x_d = nc.dram_tensor("x", [T, H], F32, kind="ExternalInput")
    gw_d = nc.dram_tensor("gw", [E, H], F32, kind="ExternalInput")
    gb_d = nc.dram_tensor("gb", [1, E], F32, kind="ExternalInput")
    wsel_d = nc.dram_tensor("wsel", [E, E_LOC], F32, kind="ExternalInput")
    eg_d = nc.dram_tensor("eg", [E_LOC, I, H], F32, kind="ExternalInput")
    eu_d = nc.dram_tensor("eu", [E_LOC, I, H], F32, kind="ExternalInput")
    ed_d = nc.dram_tensor("ed", [E_LOC, H, I], F32, kind="ExternalInput")
    sg_d = nc.dram_tensor("sg", [ISH, H], F32, kind="ExternalInput")
    su_d = nc.dram_tensor("su", [ISH, H], F32, kind="ExternalInput")
    sd_d = nc.dram_tensor("sd", [H, ISH], F32, kind="ExternalInput")
    out_rows = T // num_devices if use_collective else T
    out_d = nc.dram_tensor("out", [out_rows, H], F32, kind="ExternalOutput")

    with tile.TileContext(nc) as tc, ExitStack() as ctx:
        const = ctx.enter_context(tc.tile_pool(name="const", bufs=1))
        sbr = ctx.enter_context(tc.tile_pool(name="router", bufs=2))
        xpool = ctx.enter_context(tc.tile_pool(name="xt", bufs=1))
        wg_pool = ctx.enter_context(tc.tile_pool(name="wgt", bufs=8))
        wu_pool = ctx.enter_context(tc.tile_pool(name="wut", bufs=8))
        wd_pool = ctx.enter_context(tc.tile_pool(name="wdt", bufs=4))
        nat_pool = ctx.enter_context(tc.tile_pool(name="nat", bufs=3))
        ats_pool = ctx.enter_context(tc.tile_pool(name="ats", bufs=4))
        tmp_pool = ctx.enter_context(tc.tile_pool(name="tmp", bufs=2))
        acc_pool = ctx.enter_context(tc.tile_pool(name="acc", bufs=1))
        dram = ctx.enter_context(tc.tile_pool(name="dram", bufs=1, space="DRAM"))

        ps_a = ctx.enter_context(tc.tile_pool(name="ps_a", bufs=3, space="PSUM"))
        ps_s = ctx.enter_context(tc.tile_pool(name="ps_s", bufs=3, space="PSUM"))
        ps_o = ctx.enter_context(tc.tile_pool(name="ps_o", bufs=2, space="PSUM"))

        ident_f = const.tile([128, 128], F32, name="ident_f")
        make_identity(nc, ident_f[:])
        ident_r = const.tile([128, 128], F32R, name="ident_r")
        nc.vector.tensor_copy(ident_r[:], ident_f[:])

        # ------------- router prep ------------------------------------------
        gw_sb = nat_pool.tile([E, H], F32, name="gw_sb", tag="xn", bufs=2)
        nc.gpsimd.dma_start(gw_sb[:], gw_d[:])
        gb_sb = sbr.tile([1, E], F32, name="gb_sb")
        nc.gpsimd.dma_start(gb_sb[:], gb_d[:])
        wsel_sb = sbr.tile([E, E_LOC], F32R, name="wsel_sb")
        nc.gpsimd.dma_start(wsel_sb[:], wsel_d[:].bitcast(F32R))

        gwT = [sbr.tile([128, E], F32, name=f"gwT{h}") for h in range(HK)]
        for ht in range(HK):
            p = ps_a.tile([128, 512], F32, name=f"gwp{ht}", tag="ps_a")
            nc.tensor.transpose(p[:, 0:E], gw_sb[:, ht * 128:(ht + 1) * 128],
                                ident_f[0:E, 0:E])
            nc.vector.tensor_copy(gwT[ht][:], p[:, 0:E])

        ones_f = const.tile([1, 128], F32, name="ones_f")
        nc.vector.memset(ones_f[:], 1.0)
        bias_bc = sbr.tile([128, E], F32, name="bias_bc")
        pb = ps_a.tile([128, 512], F32, name="pb", tag="ps_a")
        nc.tensor.matmul(pb[:, 0:E], ones_f[:], gb_sb[:], start=True, stop=True)
        nc.vector.tensor_copy(bias_bc[:], pb[:, 0:E])

        # ------------- persistent activations / accumulators -----------------
        xT_r = xpool.tile([128, HK * T], F32R, name="xTr")
        xview = xT_r[:].rearrange("p (h t) -> p h t", h=HK)
        wT_r = sbr.tile([E, T], F32R, name="wT_r")
        lgs = [sbr.tile([128, E], F32, name=f"lgs{tt}") for tt in range(TT)]
        out_acc = [acc_pool.tile([128, H], F32, name=f"oacc{tt}")
                   for tt in range(TT)]

        # ------------- weight prep (load + transpose), batched ---------------
        def make_prep(ei, kind, idx, ike):
            """Allocate destination tiles; return (wgTd, wdT, batches) where
            each batch closure emits a couple of PE transposes + one copy."""
            wgTd = {}
            batches = []
            for gu in ("g", "u"):
                pool = wg_pool if gu == "g" else wu_pool
                tagw = "wgt" if gu == "g" else "wut"
                dst = [pool.tile([128, ike * 128], F32R,
                                 name=f"w{gu}T{ei}_{h}", tag=tagw)
                       for h in range(HK)]
                wgTd[gu] = dst
                ihalf = 2 if kind == "expert" else 1
                per = ike // ihalf
                for ih in range(ihalf):
                    state = {}

                    def load_half(gu=gu, ih=ih, per=per, state=state):
                        natm = [nat_pool.tile([128, H], F32R,
                                              name=f"nat{ei}{gu}{ih}_{i}",
                                              tag="natgu", bufs=4)
                                for i in range(per)]
                        state["natm"] = natm
                        for i in range(per):
                            it = ih * per + i
                            if kind == "expert":
                                mat = eg_d if gu == "g" else eu_d
                                src = mat[idx, it * 128:(it + 1) * 128, :]
                            else:
                                mat = sg_d if gu == "g" else su_d
                                src = mat[it * 128:(it + 1) * 128, :]
                            nc.sync.dma_start(natm[i][:], src.bitcast(F32R))

                    for ht in range(HK):
                        def b(gu=gu, ih=ih, ht=ht, per=per, state=state,
                              dst=dst, lh=load_half):
                            if ht == 0:
                                lh()
                            natm = state["natm"]
                            p = ps_a.tile([128, 512], F32,
                                          name=f"wp{ei}{gu}{ih}{ht}", tag="ps_a")
                            for i in range(per):
                                nc.tensor.transpose(
                                    p[:, i * 128:(i + 1) * 128].bitcast(F32R),
                                    natm[i][:, ht * 128:(ht + 1) * 128],
                                    ident_r[:])
                            nc.scalar.copy(
                                dst[ht][:, ih * per * 128:(ih + 1) * per * 128],
                                p[:, 0:per * 128].bitcast(F32R))
                        batches.append(b)

            wdT = [wd_pool.tile([128, H], F32R, name=f"wdT{ei}_{i}", tag="wdt")
                   for i in range(ike)]
            for htq in range(HK // 4):
                state = {}

                def load_dn(htq=htq, state=state):
                    dn_nat = [nat_pool.tile([128, ike * 128], F32R,
                                            name=f"dnat{ei}_{htq}_{j}",
                                            tag="natdn", bufs=6)
                              for j in range(4)]
                    state["dn"] = dn_nat
                    for j in range(4):
                        ht = htq * 4 + j
                        if kind == "expert":
                            src = ed_d[idx, ht * 128:(ht + 1) * 128, :]
                        else:
                            src = sd_d[ht * 128:(ht + 1) * 128, :]
                        nc.sync.dma_start(dn_nat[j][:], src.bitcast(F32R))

                for it in range(ike):
                    def b(htq=htq, it=it, state=state, wdT=wdT, ld=load_dn):
                        if it == 0:
                            ld()
                        dn_nat = state["dn"]
                        p = ps_a.tile([128, 512], F32,
                                      name=f"dp{ei}_{htq}_{it}", tag="ps_a")
                        for j in range(4):
                            nc.tensor.transpose(
                                p[:, j * 128:(j + 1) * 128].bitcast(F32R),
                                dn_nat[j][:, it * 128:(it + 1) * 128],
                                ident_r[:])
                        nc.vector.tensor_copy(
                            wdT[it][:, htq * 512:(htq + 1) * 512].bitcast(F32R),
                            p[:].bitcast(F32R))
                    batches.append(b)
            return wgTd, wdT, batches

        # ------------- stage 1 / stage 3 -------------------------------------
        def stage1(ei, kind, ike, wgTd, wb):
            aTs = [ats_pool.tile([128, T], F32R, name=f"aTs{ei}_{i}", tag="ats")
                   for i in range(ike)]
            for th in range(TH):
                for it in range(ike):
                    gp = ps_s.tile([128, 512], F32, name=f"gp{ei}_{it}_{th}",
                                   tag="ps_s")
                    up = ps_s.tile([128, 512], F32, name=f"up{ei}_{it}_{th}",
                                   tag="ps_s")
                    for ht in range(HK):
                        nc.tensor.matmul(
                            gp[:], wgTd["g"][ht][:, it * 128:(it + 1) * 128],
                            xT_r[:, ht * T + th * 512: ht * T + (th + 1) * 512],
                            start=(ht == 0), stop=(ht == HK - 1))
                    for ht in range(HK):
                        nc.tensor.matmul(
                            up[:], wgTd["u"][ht][:, it * 128:(it + 1) * 128],
                            xT_r[:, ht * T + th * 512: ht * T + (th + 1) * 512],
                            start=(ht == 0), stop=(ht == HK - 1))
                    sg_t = tmp_pool.tile([128, 512], F32, name=f"sl{ei}{it}{th}",
                                         tag="silu", bufs=3)
                    nc.scalar.activation(sg_t[:], gp[:], AF.Silu)
                    dst = aTs[it][:, th * 512:(th + 1) * 512].bitcast(F32R)
                    if kind == "expert":
                        nc.vector.tensor_tensor(sg_t[:], sg_t[:], up[:], OP.mult)
                        nc.vector.tensor_tensor(
                            dst, sg_t[:], wb[:, th * 512:(th + 1) * 512],
                            OP.mult)
                    else:
                        nc.vector.tensor_tensor(dst, sg_t[:], up[:], OP.mult)
            return aTs

        def stage3_groups(ei, ike, aTs, wdT):
            groups = []
            for tt in range(TT):
                for hh in range(NH):
                    def g(tt=tt, hh=hh):
                        op = ps_o.tile([128, 512], F32, name=f"op{ei}_{tt}_{hh}",
                                       tag="ps_o")
                        for it in range(ike):
                            nc.tensor.matmul(
                                op[:], aTs[it][:, tt * 128:(tt + 1) * 128],
                                wdT[it][:, hh * 512:(hh + 1) * 512],
                                start=(it == 0), stop=(it == ike - 1))
                        dst = out_acc[tt][:, hh * 512:(hh + 1) * 512]
                        if ei == 0:
                            nc.vector.tensor_copy(dst, op[:])
                        else:
                            nc.vector.tensor_tensor(dst, dst, op[:], OP.add)
                    groups.append(g)
            return groups

        def emit_interleaved(groups, batches):
            bi = 0
            n = len(groups)
            for gi, g in enumerate(groups):
                g()
                # front-load: finish all batches by ~2/3 through the groups
                want = min(len(batches), (gi + 1) * len(batches) * 2 // n)
                while bi < want:
                    batches[bi]()
                    bi += 1
            while bi < len(batches):
                batches[bi]()
                bi += 1

        # ------------- routing ------------------------------------------------
        def do_routing(tt):
            S = sbr.tile([128, E], F32, name=f"S{tt}", tag="S")
            nc.scalar.activation(S[:], lgs[tt][:], AF.Sigmoid)
            SC = sbr.tile([128, E], F32, name=f"SC{tt}", tag="SC")
            nc.vector.tensor_tensor(SC[:], S[:], bias_bc[:], OP.add)
            topg = sbr.tile([128, E], F32, name=f"topg{tt}", tag="topg")
            for g in range(4):
                nc.vector.max(topg[:, 8 * g:8 * g + 8], SC[:, 8 * g:8 * g + 8])
            gs8 = sbr.tile([128, 8], F32, name=f"gs8{tt}", tag="gs8")
            nc.vector.memset(gs8[:], -1e30)
            tg = topg[:].rearrange("p (g k) -> p g k", k=8)
            nc.vector.tensor_tensor(gs8[:, 0:4], tg[:, :, 0], tg[:, :, 1], OP.add)
            gtop = sbr.tile([128, 8], F32, name=f"gtop{tt}", tag="gtop")
            nc.vector.max(gtop[:], gs8[:])
            gmask = sbr.tile([128, 4], F32, name=f"gmask{tt}", tag="gmask")
            nc.vector.tensor_scalar(gmask[:], gs8[:, 0:4], gtop[:, 1:2], None,
                                    OP.is_ge)
            SCm = sbr.tile([128, E], F32, name=f"SCm{tt}", tag="SCm")
            nc.vector.tensor_tensor(
                SCm[:].rearrange("p (g k) -> p g k", k=8),
                SC[:].rearrange("p (g k) -> p g k", k=8),
                gmask[:].rearrange("p (g k) -> p g k", k=1).broadcast_to(
                    [128, 4, 8]),
                OP.mult)
            etop = sbr.tile([128, 8], F32, name=f"etop{tt}", tag="etop")
            nc.vector.max(etop[:], SCm[:])
            sel = sbr.tile([128, E], F32, name=f"sel{tt}", tag="sel")
            nc.vector.tensor_scalar(sel[:], SCm[:], etop[:, 7:8], None, OP.is_ge)
            wr = sbr.tile([128, E], F32, name=f"wr{tt}", tag="wr")
            nc.vector.tensor_tensor(wr[:], S[:], sel[:], OP.mult)
            den = sbr.tile([128, 1], F32, name=f"den{tt}", tag="den")
            nc.vector.reduce_sum(den[:], wr[:], axis=AX.X)
            nc.vector.tensor_scalar(den[:], den[:], 1.0 / 2.5, None, OP.mult)
            dinv = sbr.tile([128, 1], F32, name=f"dinv{tt}", tag="dinv")
            nc.vector.reciprocal(dinv[:], den[:])
            wt = sbr.tile([128, E], F32, name=f"wt{tt}", tag="wt")
            nc.vector.tensor_scalar(wt[:], wr[:], dinv[:], None, OP.mult)
            tp = ps_a.tile([128, 512], F32, name=f"tw{tt}", tag="ps_a")
            nc.tensor.transpose(tp[0:E, 0:128], wt[:], ident_f[:])
            nc.vector.tensor_copy(wT_r[:, tt * 128:(tt + 1) * 128].bitcast(F32R),
                                  tp[0:E, 0:128].bitcast(F32R))

        wb_tiles = {}
        wsel_bcs = {}

        def wb_th(e, th):
            if e not in wsel_bcs:
                wselbc = tmp_pool.tile([E, 128], F32R, name=f"wsb{e}",
                                       tag="wselbc")
                nc.vector.tensor_copy(
                    wselbc[:], wsel_sb[:, e:e + 1].broadcast_to([E, 128]))
                wsel_bcs[e] = wselbc
            if e not in wb_tiles:
                wb_tiles[e] = tmp_pool.tile([128, T], F32, name=f"wbx{e}",
                                            tag="wb")
            wb = wb_tiles[e]
            p = ps_a.tile([128, 512], F32, name=f"wbp{e}_{th}", tag="ps_a")
            nc.tensor.matmul(p[:], wsel_bcs[e][:],
                             wT_r[:, th * 512:(th + 1) * 512],
                             start=True, stop=True)
            nc.vector.tensor_copy(wb[:, th * 512:(th + 1) * 512], p[:])

        def make_wb(e):
            wb_th(e, 0)
            wb_th(e, 1)

        # ================= emission schedule =================================
        # Phase X: x transposes + router logits, with expert-0's weight prep
        # interleaved.  Routing is split by token half so expert-0's stage-1
        # th=0 can start as soon as tokens 0..511 are routed.  The shared
        # entry runs LAST so its (pool-serialized) weight prep overlaps the
        # final expert instead of the congested startup window.
        e0_prep = make_prep(0, "expert", 0, IK)
        bi = 0
        for tt in range(TT):
            xn = nat_pool.tile([128, H], F32, name=f"xn{tt}", tag="xn", bufs=2)
            (nc.sync if tt % 2 == 0 else nc.gpsimd).dma_start(
                xn[:], x_d[tt * 128:(tt + 1) * 128, :])
            xfb = tmp_pool.tile([128, H], F32, name=f"xfb{tt}", tag="xfb")
            for hq in range(HK // 4):
                p = ps_s.tile([128, 512], F32, name=f"xp{tt}_{hq}", tag="ps_s")
                for j in range(4):
                    ht = hq * 4 + j
                    nc.tensor.transpose(
                        p[:, j * 128:(j + 1) * 128],
                        xn[:, ht * 128:(ht + 1) * 128], ident_f[:])
                nc.scalar.copy(xfb[:, hq * 512:(hq + 1) * 512], p[:])
                nc.gpsimd.tensor_copy(
                    xview[:, hq * 4:(hq + 1) * 4,
                          tt * 128:(tt + 1) * 128].bitcast(F32R),
                    xfb[:, hq * 512:(hq + 1) * 512]
                    .rearrange("p (h t) -> p h t", h=4).bitcast(F32R))

            lg = ps_a.tile([128, 512], F32, name=f"lg{tt}", tag="ps_a")
            for ht in range(HK):
                nc.tensor.matmul(lg[:, 0:E], xfb[:, ht * 128:(ht + 1) * 128],
                                 gwT[ht][:],
                                 start=(ht == 0), stop=(ht == HK - 1))
            nc.scalar.copy(lgs[tt][:], lg[:, 0:E])

            want = (tt + 1) * len(e0_prep[2]) // TT
            while bi < want:
                e0_prep[2][bi]()
                bi += 1

        for tt in range(TT // 2):
            do_routing(tt)
        wb_th(0, 0)
        wb_th(1, 0)
        for tt in range(TT // 2, TT):
            do_routing(tt)
        wb_th(0, 1)
        wb_th(1, 1)

        # Entry pipeline: stage-3 of entry k interleaves entry k+1's prep.
        order = [("expert", e, IK) for e in range(E_LOC - 1)] + \
                [("shared", 0, SK), ("expert", E_LOC - 1, IK)]
        prev = e0_prep
        for k, (kind, idx, ike) in enumerate(order):
            wgTd, wdT, _ = prev
            wb = wb_tiles.get(idx) if kind == "expert" else None
            aTs = stage1(k, kind, ike, wgTd, wb)
            if kind == "expert" and idx + 2 < E_LOC:
                make_wb(idx + 2)
            if k + 1 < len(order):
                knd, nidx, nike = order[k + 1]
                nxt = make_prep(k + 1, knd, nidx, nike)
            else:
                nxt = None
            emit_interleaved(stage3_groups(k, ike, aTs, wdT),
                             nxt[2] if nxt else [])
            prev = nxt

        # ------------- ReduceScatter + output -------------------------------
        # Each core keeps its 128-token shard of the summed output; the host
        # concatenates the 8 shards.  RS moves ~30% less wire traffic than an
        # AllReduce of the full [T, H].
        if use_collective:
            bin_t = dram.tile([T, H], F32, name="rsin")
            bout_t = dram.tile([out_rows, H], F32, name="rsout")
            for tt in range(TT):
                nc.sync.dma_start(bin_t[tt * 128:(tt + 1) * 128, :],
                                  out_acc[tt][:])
            nc.gpsimd.collective_compute(
                "ReduceScatter", OP.add,
                replica_groups=[list(range(num_devices))],
                ins=[bin_t.opt()], outs=[bout_t.opt()])
            nc.sync.dma_start(out_d[:], bout_t[:])
        else:
            for tt in range(TT):
                nc.sync.dma_start(out_d[tt * 128:(tt + 1) * 128, :],
                                  out_acc[tt][:])
    nc.compile()
    return nc


_NC_CACHE = {}


def _get_module():
    key = "spmd"
    if key not in _NC_CACHE:
        _NC_CACHE[key] = build_module(use_collective=True, num_devices=N_CORES)
    return _NC_CACHE[key]


def make_in_maps(hidden_states, gate_w, gate_bias, expert_gate, expert_up,
                 expert_down, shared_gate, shared_up, shared_down):
    x = np.ascontiguousarray(
        np.asarray(hidden_states, np.float32).reshape(T, H))
    gw = np.ascontiguousarray(np.asarray(gate_w, np.float32))
    gb = np.ascontiguousarray(np.asarray(gate_bias, np.float32).reshape(1, E))
    in_maps = []
    for c in range(N_CORES):
        lo, hi = c * E_LOC, (c + 1) * E_LOC
        sel = np.zeros((E, E_LOC), np.float32)
        for j in range(E_LOC):
            sel[lo + j, j] = 1.0
        in_maps.append({
            "x": x, "gw": gw, "gb": gb, "wsel": sel,
            "eg": np.ascontiguousarray(np.asarray(expert_gate, np.float32)[lo:hi]),
            "eu": np.ascontiguousarray(np.asarray(expert_up, np.float32)[lo:hi]),
            "ed": np.ascontiguousarray(np.asarray(expert_down, np.float32)[lo:hi]),
            "sg": np.ascontiguousarray(
                np.asarray(shared_gate, np.float32)[c * ISH:(c + 1) * ISH]),
            "su": np.ascontiguousarray(
                np.asarray(shared_up, np.float32)[c * ISH:(c + 1) * ISH]),
            "sd": np.ascontiguousarray(
                np.asarray(shared_down, np.float32)[:, c * ISH:(c + 1) * ISH]),
        })
    return in_maps


def kernel(hidden_states, gate_w, gate_bias, expert_gate, expert_up,
           expert_down, shared_gate, shared_up, shared_down):
    import os
    # The axon NTFF trace hook is absent in this container; make sure the
    # PJRT execute path never tries to use it.
    os.environ.setdefault("BASS_NEVER_TRACE", "1")
    from concourse.bass_utils import run_bass_kernel_spmd
    nc = _get_module()
    in_maps = make_in_maps(hidden_states, gate_w, gate_bias, expert_gate,
                           expert_up, expert_down, shared_gate, shared_up,
                           shared_down)
    res = run_bass_kernel_spmd(nc, in_maps, core_ids=list(range(N_CORES)))
    out = np.concatenate([np.asarray(res.results[c]["out"], np.float32)
                          for c in range(N_CORES)], axis=0)
    return out.reshape(np.asarray(hidden_states).shape)



# revision 13
# speedup vs baseline: 1.2672x; 1.2672x over previous
# DeepseekV3MoECalibrate Trainium2 kernel (8 NeuronCores, expert-parallel).
#
# Sharding: 32 experts -> 4 per core; shared expert split along the 2I=2048
# intermediate dim (256 rows per core); tokens replicated; partial outputs
# summed with an on-device ReduceScatter.
#
# All weights and the token matrix are pre-transposed AND pre-packed on the
# HOST into the exact [128, free] SBUF layouts the PE needs, fp16, so the
# TensorEngine runs nothing but full-rate fp16 matmuls (no on-device
# transposes, no weight PSUM-evacuation copies) and every weight matrix is
# a single large DMA (per-DMA queue overhead ~0.9us makes small transfers
# expensive).  Router logits are computed exactly from an fp16 hi/lo split
# of x and gate_w (x.gw = xh.gh + xh.gl + xl.gh, error ~1e-7), so top-k
# selection matches the fp32 reference; the rest of the router is fp32 on
# DVE/Act.
#
# Routing weights are applied to the stage-1 activations with a deferred
# in-place scale pass on the Pool engine, so the stage-3 down-projection is
# one 18-matmul PSUM accumulation chain per output tile (4 experts x 4
# i-tiles + 2 shared i-tiles) with a single evacuation.
from contextlib import ExitStack

import numpy as np

import concourse.bass as bass
import concourse.tile as tile
from concourse import bacc, mybir
from concourse.masks import make_identity

F32 = mybir.dt.float32
F32R = mybir.dt.float32r
F16 = mybir.dt.float16
AF = mybir.ActivationFunctionType
OP = mybir.AluOpType
AX = mybir.AxisListType

N_CORES = 8
T, H, I, E = 1024, 1024, 512, 32
E_LOC = E // N_CORES          # 4 experts per core
ISH = 2 * I // N_CORES        # 256 shared-intermediate rows per core
TT = T // 128                 # 8 token tiles
HK = H // 128                 # 8 h k-tiles
IK = I // 128                 # 4 i-tiles per expert
SK = ISH // 128               # 2 shared i-tiles
TH = T // 512                 # 2 t halves (stage-1 rhs width)
NH = H // 512                 # 2 h halves (stage-3 rhs width)

# entry table: (kind, expert idx or None, #i-tiles); shared first so phase A
# can start before the router finishes (no routing weight needed).
ENTRIES = [("shared", None, SK)] + [("expert", e, IK) for e in range(E_LOC)]
N_ITILES = SK + E_LOC * IK    # 18 i-tiles total


def build_module(use_collective=True, num_devices=N_CORES):
    nc = bacc.Bacc("TRN2", target_bir_lowering=False, debug=False,
                   num_devices=num_devices)

    xh_d = nc.dram_tensor("xh", [H, T], F16, kind="ExternalInput")
    xl_d = nc.dram_tensor("xl", [H, T], F16, kind="ExternalInput")
    # gh/gl packed: [128, (ht, {gh,gl}, E)]
    ghl_d = nc.dram_tensor("ghl", [128, HK * 2 * E], F16, kind="ExternalInput")
    bias_d = nc.dram_tensor("bias", [128, E], F32, kind="ExternalInput")
    wselbc_d = nc.dram_tensor("wselbc", [E, E_LOC * 128], F32,
                              kind="ExternalInput")
    # per-expert gate/up packed [128, (ht, I)]; down packed [128, (it, H)]
    wg_d = nc.dram_tensor("wg", [E_LOC, 128, HK * I], F16,
                          kind="ExternalInput")
    wu_d = nc.dram_tensor("wu", [E_LOC, 128, HK * I], F16,
                          kind="ExternalInput")
    wd_d = nc.dram_tensor("wd", [E_LOC, 128, IK * H], F16,
                          kind="ExternalInput")
    sg_d = nc.dram_tensor("sg", [128, HK * ISH], F16, kind="ExternalInput")
    su_d = nc.dram_tensor("su", [128, HK * ISH], F16, kind="ExternalInput")
    sd_d = nc.dram_tensor("sd", [128, SK * H], F16, kind="ExternalInput")
    out_rows = T // num_devices if use_collective else T
    out_d = nc.dram_tensor("out", [out_rows, H], F32, kind="ExternalOutput")

    with tile.TileContext(nc) as tc, ExitStack() as ctx:
        const = ctx.enter_context(tc.tile_pool(name="const", bufs=1))
        sbr = ctx.enter_context(tc.tile_pool(name="router", bufs=2))
        xpool = ctx.enter_context(tc.tile_pool(name="xt", bufs=1))
        xlp = ctx.enter_context(tc.tile_pool(name="xl", bufs=1))
        wgu_pool = ctx.enter_context(tc.tile_pool(name="wgu", bufs=1))
        wd_pool = ctx.enter_context(tc.tile_pool(name="wd", bufs=1))
        a_pool = ctx.enter_context(tc.tile_pool(name="ats", bufs=1))
        wb_pool = ctx.enter_context(tc.tile_pool(name="wb", bufs=1))
        tmp_pool = ctx.enter_context(tc.tile_pool(name="tmp", bufs=3))
        stg_pool = ctx.enter_context(tc.tile_pool(name="stg", bufs=3))
        dram = ctx.enter_context(tc.tile_pool(name="dram", bufs=1, space="DRAM"))

        ps_main = ctx.enter_context(tc.tile_pool(name="ps_main", bufs=4,
                                                 space="PSUM"))
        ps_r = ctx.enter_context(tc.tile_pool(name="ps_r", bufs=3,
                                              space="PSUM"))
        ps_lg = ctx.enter_context(tc.tile_pool(name="ps_lg", bufs=1,
                                               space="PSUM"))

        ident_f = const.tile([128, 128], F32, name="ident_f")
        make_identity(nc, ident_f[:])

        # ---- DMA plan ------------------------------------------------------
        # The DMA engines serve all queues as ONE serial stream (~344 GB/s),
        # so everything goes on the sync/HWDGE queue in exact consumption
        # order: shared g, x tiles (pace the first chains), shared u, gate
        # table, e0 weights, xl stream, router smalls, e1..e3 weights, down
        # weights, outputs.
        sg_sb = wgu_pool.tile([128, HK * ISH], F16, name="sg_sb")
        nc.sync.dma_start(sg_sb[:], sg_d[:])
        su_sb = wgu_pool.tile([128, HK * ISH], F16, name="su_sb")
        nc.sync.dma_start(su_sb[:], su_d[:])
        ghl_sb = sbr.tile([128, HK * 2 * E], F16, name="ghl_sb")
        nc.sync.dma_start(ghl_sb[:], ghl_d[:])
        xt = [xpool.tile([128, T], F16, name=f"xt{ht}") for ht in range(HK)]
        for ht in range(HK):
            nc.sync.dma_start(xt[ht][:], xh_d[ht * 128:(ht + 1) * 128, :])

        wg_sb, wu_sb = [sg_sb], [su_sb]
        for e in range(E_LOC):
            g = wgu_pool.tile([128, HK * I], F16, name=f"wg{e}")
            u = wgu_pool.tile([128, HK * I], F16, name=f"wu{e}")
            if e == 0:
                nc.sync.dma_start(g[:], wg_d[e])
                nc.sync.dma_start(u[:], wu_d[e])
            wg_sb.append(g)
            wu_sb.append(u)

        wd_sb = [wd_pool.tile([128, SK * H], F16, name="sd_sb")]
        for e in range(E_LOC):
            wd_sb.append(wd_pool.tile([128, IK * H], F16, name=f"wd{e}"))

        def late_dmas():
            # issued after the xl stream in queue order
            for e in range(1, E_LOC):
                nc.sync.dma_start(wg_sb[1 + e][:], wg_d[e])
                nc.sync.dma_start(wu_sb[1 + e][:], wu_d[e])
            nc.sync.dma_start(wd_sb[0][:], sd_d[:])
            for e in range(E_LOC):
                nc.sync.dma_start(wd_sb[1 + e][:], wd_d[e])

        bias_bc = sbr.tile([128, E], F32, name="bias_bc")
        wselbc_sb = sbr.tile([E, E_LOC * 128], F32R, name="wselbc_sb")

        a_tiles = [a_pool.tile([128, T], F16, name=f"a{i}")
                   for i in range(N_ITILES)]
        a_base = {}
        off = 0
        for ei, (kind, e, ike) in enumerate(ENTRIES):
            a_base[ei] = off
            off += ike

        # ---- router: exact fp16-split logits -------------------------------
        lgall = ps_lg.tile([128, TT * E], F32, name="lgall")

        def gh_sl(ht):
            return ghl_sb[:, ht * 2 * E:ht * 2 * E + E]

        def gl_sl(ht):
            return ghl_sb[:, ht * 2 * E + E:(ht + 1) * 2 * E]

        def logits12_group(ht):
            # xh.gh + xh.gl terms (no xl dependency).  PSUM start_tensor_calc
            # marks the whole 2KB zero region pending-zero, so ONLY the very
            # first matmul into lgall may set start=True: every slice's first
            # touch then auto-zeroes, later touches accumulate.
            for pi, rh in enumerate((gh_sl(ht), gl_sl(ht))):
                for tt in range(TT):
                    nc.tensor.matmul(
                        lgall[:, tt * E:(tt + 1) * E],
                        xt[ht][:, tt * 128:(tt + 1) * 128],
                        rh,
                        start=(ht == 0 and pi == 0 and tt == 0), stop=False,
                        skip_group_check=True)

        def logits3_group(ht):
            # xl.gh correction term
            xlt = xlp.tile([128, T], F16, name=f"xl{ht}", tag="xl", bufs=5)
            nc.sync.dma_start(xlt[:], xl_d[ht * 128:(ht + 1) * 128, :])
            for tt in range(TT):
                nc.tensor.matmul(
                    lgall[:, tt * E:(tt + 1) * E],
                    xlt[:, tt * 128:(tt + 1) * 128],
                    gh_sl(ht),
                    start=False, stop=(ht == HK - 1),
                    skip_group_check=True)

        # ---- stage 1 for the shared entry: th=0 runs ht-outer across all
        # four PSUM chains so the PE keeps pace with the arriving xt tiles
        # (one matmul per chain per tile) instead of idling on the first
        # chain; logits groups slot in per-ht as extra filler.
        def stage_a0(interleave_ht):
            ike = SK
            ab = a_tiles[0:SK]
            gps = [ps_main.tile([128, 512], F32, name=f"gp0_0_{it}", tag="ps")
                   for it in range(ike)]
            ups = [ps_main.tile([128, 512], F32, name=f"up0_0_{it}", tag="ps")
                   for it in range(ike)]
            for ht in range(HK):
                for it in range(ike):
                    nc.tensor.matmul(
                        gps[it][:],
                        sg_sb[:, (ht * ike + it) * 128:(ht * ike + it + 1) * 128],
                        xt[ht][:, 0:512],
                        start=(ht == 0), stop=(ht == HK - 1))
                    nc.tensor.matmul(
                        ups[it][:],
                        su_sb[:, (ht * ike + it) * 128:(ht * ike + it + 1) * 128],
                        xt[ht][:, 0:512],
                        start=(ht == 0), stop=(ht == HK - 1))
                interleave_ht(ht)
            for it in range(ike):
                sg_t = tmp_pool.tile([128, 512], F32, name=f"sl0_0_{it}",
                                     tag="silu")
                nc.scalar.activation(sg_t[:], gps[it][:], AF.Silu)
                nc.vector.tensor_tensor(ab[it][:, 0:512], sg_t[:], ups[it][:],
                                        OP.mult)
            for it in range(ike):
                gp = ps_main.tile([128, 512], F32, name=f"gp0_1_{it}",
                                  tag="ps")
                up = ps_main.tile([128, 512], F32, name=f"up0_1_{it}",
                                  tag="ps")
                for ht in range(HK):
                    nc.tensor.matmul(
                        gp[:],
                        sg_sb[:, (ht * ike + it) * 128:(ht * ike + it + 1) * 128],
                        xt[ht][:, 512:1024],
                        start=(ht == 0), stop=(ht == HK - 1))
                for ht in range(HK):
                    nc.tensor.matmul(
                        up[:],
                        su_sb[:, (ht * ike + it) * 128:(ht * ike + it + 1) * 128],
                        xt[ht][:, 512:1024],
                        start=(ht == 0), stop=(ht == HK - 1))
                sg_t = tmp_pool.tile([128, 512], F32, name=f"sl0_1_{it}",
                                     tag="silu")
                nc.scalar.activation(sg_t[:], gp[:], AF.Silu)
                nc.vector.tensor_tensor(ab[it][:, 512:1024], sg_t[:], up[:],
                                        OP.mult)

        # ---- stage 1 (gate/up chains + silu*up into aTs) -------------------
        def stage_a(ei, interleave=None):
            kind, e, ike = ENTRIES[ei]
            wgt, wut = wg_sb[ei], wu_sb[ei]
            ab = a_tiles[a_base[ei]:a_base[ei] + ike]
            step = 0
            for th in range(TH):
                for it in range(ike):
                    gp = ps_main.tile([128, 512], F32,
                                      name=f"gp{ei}_{th}_{it}", tag="ps")
                    up = ps_main.tile([128, 512], F32,
                                      name=f"up{ei}_{th}_{it}", tag="ps")
                    for ht in range(HK):
                        nc.tensor.matmul(
                            gp[:],
                            wgt[:, (ht * ike + it) * 128:(ht * ike + it + 1) * 128],
                            xt[ht][:, th * 512:(th + 1) * 512],
                            start=(ht == 0), stop=(ht == HK - 1))
                    for ht in range(HK):
                        nc.tensor.matmul(
                            up[:],
                            wut[:, (ht * ike + it) * 128:(ht * ike + it + 1) * 128],
                            xt[ht][:, th * 512:(th + 1) * 512],
                            start=(ht == 0), stop=(ht == HK - 1))
                    sg_t = tmp_pool.tile([128, 512], F32,
                                         name=f"sl{ei}_{th}_{it}", tag="silu")
                    nc.scalar.activation(sg_t[:], gp[:], AF.Silu)
                    nc.vector.tensor_tensor(
                        ab[it][:, th * 512:(th + 1) * 512],
                        sg_t[:], up[:], OP.mult)
                    if interleave is not None:
                        interleave(step)
                    step += 1

        # ---- router top-k math (DVE/Act only; transposes deferred) --------
        wt_tiles = []

        def routing_math(tt):
            lg = lgall[:, tt * E:(tt + 1) * E]
            S = sbr.tile([128, E], F32, name=f"S{tt}", tag="S")
            nc.scalar.activation(S[:], lg, AF.Sigmoid)
            SC = sbr.tile([128, E], F32, name=f"SC{tt}", tag="SC")
            nc.vector.tensor_tensor(SC[:], S[:], bias_bc[:], OP.add)
            topg = sbr.tile([128, E], F32, name=f"topg{tt}", tag="topg")
            for g in range(4):
                nc.vector.max(topg[:, 8 * g:8 * g + 8], SC[:, 8 * g:8 * g + 8])
            gs8 = sbr.tile([128, 8], F32, name=f"gs8{tt}", tag="gs8")
            nc.vector.memset(gs8[:], -1e30)
            tg = topg[:].rearrange("p (g k) -> p g k", k=8)
            nc.vector.tensor_tensor(gs8[:, 0:4], tg[:, :, 0], tg[:, :, 1],
                                    OP.add)
            gtop = sbr.tile([128, 8], F32, name=f"gtop{tt}", tag="gtop")
            nc.vector.max(gtop[:], gs8[:])
            gmask = sbr.tile([128, 4], F32, name=f"gmask{tt}", tag="gmask")
            nc.vector.tensor_scalar(gmask[:], gs8[:, 0:4], gtop[:, 1:2], None,
                                    OP.is_ge)
            SCm = sbr.tile([128, E], F32, name=f"SCm{tt}", tag="SCm")
            nc.vector.tensor_tensor(
                SCm[:].rearrange("p (g k) -> p g k", k=8),
                SC[:].rearrange("p (g k) -> p g k", k=8),
                gmask[:].rearrange("p (g k) -> p g k", k=1).broadcast_to(
                    [128, 4, 8]),
                OP.mult)
            etop = sbr.tile([128, 8], F32, name=f"etop{tt}", tag="etop")
            nc.vector.max(etop[:], SCm[:])
            sel = sbr.tile([128, E], F32, name=f"sel{tt}", tag="sel")
            nc.vector.tensor_scalar(sel[:], SCm[:], etop[:, 7:8], None,
                                    OP.is_ge)
            wr = sbr.tile([128, E], F32, name=f"wr{tt}", tag="wr")
            nc.vector.tensor_tensor(wr[:], S[:], sel[:], OP.mult)
            den = sbr.tile([128, 1], F32, name=f"den{tt}", tag="den")
            nc.vector.reduce_sum(den[:], wr[:], axis=AX.X)
            nc.vector.tensor_scalar(den[:], den[:], 1.0 / 2.5, None, OP.mult)
            dinv = sbr.tile([128, 1], F32, name=f"dinv{tt}", tag="dinv")
            nc.vector.reciprocal(dinv[:], den[:])
            wt = sbr.tile([128, E], F32, name=f"wt{tt}", tag="wt", bufs=8)
            nc.vector.tensor_scalar(wt[:], wr[:], dinv[:], None, OP.mult)
            wt_tiles.append(wt)

        # ================= emission schedule ===============================
        # Shared entry first (needs no routing weights); logit terms with no
        # xl dependency slot into its 4 stage-1 steps, the xl correction term
        # into expert-0's first steps.  Router math then lands between e0 and
        # e1 stage-2 work on DVE/Act; wt transposes + wb rows run on the PE
        # right after (DVE router is done by then), so expert-1 stage-2 never
        # queues behind them.
        stage_a0(interleave_ht=logits12_group)
        stage_a(1, interleave=lambda s: (logits3_group(2 * s),
                                         logits3_group(2 * s + 1))
                if s < 4 else None)
        nc.sync.dma_start(bias_bc[:], bias_d[:])
        nc.sync.dma_start(wselbc_sb[:], wselbc_d[:].bitcast(F32R))
        late_dmas()
        for tt in range(TT):
            routing_math(tt)

        # wt transposes + routing-weight broadcast rows
        wT_r = sbr.tile([E, T], F32R, name="wT_r")
        for tt in range(TT):
            p = ps_r.tile([128, 512], F32, name=f"wtp{tt}", tag="ps_r")
            nc.tensor.transpose(p[0:E, 0:128], wt_tiles[tt][:], ident_f[:])
            nc.vector.tensor_copy(wT_r[:, tt * 128:(tt + 1) * 128].bitcast(F32R),
                                  p[0:E, 0:128].bitcast(F32R))
        wb_tiles = []
        for e in range(E_LOC):
            wbt = wb_pool.tile([128, T], F16, name=f"wb{e}")
            for th in range(TH):
                p = ps_r.tile([128, 512], F32, name=f"wbp{e}_{th}", tag="ps_r")
                nc.tensor.matmul(p[:], wselbc_sb[:, e * 128:(e + 1) * 128],
                                 wT_r[:, th * 512:(th + 1) * 512],
                                 start=True, stop=True)
                nc.vector.tensor_copy(wbt[:, th * 512:(th + 1) * 512], p[:])
            wb_tiles.append(wbt)

        # deferred routing-weight scale: in-place on the Pool engine.
        def scale_pass(ei):
            kind, e, ike = ENTRIES[ei]
            ab = a_tiles[a_base[ei]:a_base[ei] + ike]
            for th in range(TH):
                for it in range(ike):
                    sl = ab[it][:, th * 512:(th + 1) * 512]
                    nc.gpsimd.tensor_tensor(
                        sl, sl, wb_tiles[e][:, th * 512:(th + 1) * 512],
                        OP.mult)

        scale_pass(1)
        stage_a(2)
        scale_pass(2)
        for ei in range(3, len(ENTRIES)):
            stage_a(ei)
            scale_pass(ei)

        # ---- stage 3: one 18-matmul PSUM chain per output tile ------------
        if use_collective:
            bin_t = dram.tile([T, H], F32, name="rsin")
            target = bin_t
        else:
            target = out_d
        for tt in range(TT):
            for hh in range(NH):
                op = ps_main.tile([128, 512], F32, name=f"o{tt}_{hh}",
                                  tag="ps")
                n = 0
                for ei, (kind, e, ike) in enumerate(ENTRIES):
                    for it in range(ike):
                        nc.tensor.matmul(
                            op[:],
                            a_tiles[a_base[ei] + it][:, tt * 128:(tt + 1) * 128],
                            wd_sb[ei][:, it * H + hh * 512:
                                      it * H + (hh + 1) * 512],
                            start=(n == 0), stop=(n == N_ITILES - 1))
                        n += 1
                st = stg_pool.tile([128, 512], F32, name=f"st{tt}_{hh}",
                                   tag="stg")
                nc.vector.tensor_copy(st[:], op[:])
                nc.sync.dma_start(
                    target[tt * 128:(tt + 1) * 128, hh * 512:(hh + 1) * 512],
                    st[:])

        # ---- ReduceScatter + output ---------------------------------------
        if use_collective:
            bout_t = dram.tile([out_rows, H], F32, name="rsout")
            nc.gpsimd.collective_compute(
                "ReduceScatter", OP.add,
                replica_groups=[list(range(num_devices))],
                ins=[bin_t.opt()], outs=[bout_t.opt()])
            nc.sync.dma_start(out_d[:], bout_t[:])
    nc.compile()
    return nc


_NC_CACHE = {}


def _get_module():
    key = "spmd"
    if key not in _NC_CACHE:
        _NC_CACHE[key] = build_module(use_collective=True, num_devices=N_CORES)
    return _NC_CACHE[key]


def _pack_rows(a, blk=128):
    """[R, C] -> [128, (R//128) * C]: row-tile r128 layout for one-DMA loads."""
    r, c = a.shape
    return np.ascontiguousarray(
        a.reshape(r // blk, blk, c).transpose(1, 0, 2).reshape(blk, -1))


def make_in_maps(hidden_states, gate_w, gate_bias, expert_gate, expert_up,
                 expert_down, shared_gate, shared_up, shared_down):
    x = np.asarray(hidden_states, np.float32).reshape(T, H)
    xt = np.ascontiguousarray(x.T)                       # [H, T]
    xh = xt.astype(np.float16)
    xl = (xt - xh.astype(np.float32)).astype(np.float16)
    gwt = np.ascontiguousarray(np.asarray(gate_w, np.float32).T)  # [H, E]
    gh = gwt.astype(np.float16)
    gl = (gwt - gh.astype(np.float32)).astype(np.float16)
    # pack gh/gl as [128, (ht, {gh,gl}, E)]
    ghl = np.concatenate(
        [gh.reshape(HK, 128, E)[:, :, None, :],
         gl.reshape(HK, 128, E)[:, :, None, :]], axis=2)  # [HK,128,2,E]
    ghl = np.ascontiguousarray(
        ghl.transpose(1, 0, 2, 3).reshape(128, HK * 2 * E))
    bias = np.broadcast_to(
        np.asarray(gate_bias, np.float32).reshape(1, E), (128, E))
    bias = np.ascontiguousarray(bias)
    eg = np.asarray(expert_gate, np.float32)
    eu = np.asarray(expert_up, np.float32)
    ed = np.asarray(expert_down, np.float32)
    sgT = np.asarray(shared_gate, np.float32).T          # [H, 2I]
    suT = np.asarray(shared_up, np.float32).T            # [H, 2I]
    sd = np.asarray(shared_down, np.float32)             # [H, 2I]
    in_maps = []
    for c in range(N_CORES):
        lo, hi = c * E_LOC, (c + 1) * E_LOC
        wselbc = np.zeros((E, E_LOC * 128), np.float32)
        for j in range(E_LOC):
            wselbc[lo + j, j * 128:(j + 1) * 128] = 1.0
        wg = np.stack([_pack_rows(eg[lo + j].T.astype(np.float16))
                       for j in range(E_LOC)])           # [E_LOC,128,HK*I]
        wu = np.stack([_pack_rows(eu[lo + j].T.astype(np.float16))
                       for j in range(E_LOC)])
        wd = np.stack([_pack_rows(ed[lo + j].T.astype(np.float16))
                       for j in range(E_LOC)])           # [E_LOC,128,IK*H]
        in_maps.append({
            "xh": xh, "xl": xl, "ghl": ghl, "bias": bias, "wselbc": wselbc,
            "wg": wg, "wu": wu, "wd": wd,
            "sg": _pack_rows(np.ascontiguousarray(
                sgT[:, c * ISH:(c + 1) * ISH]).astype(np.float16)),
            "su": _pack_rows(np.ascontiguousarray(
                suT[:, c * ISH:(c + 1) * ISH]).astype(np.float16)),
            "sd": _pack_rows(np.ascontiguousarray(
                sd[:, c * ISH:(c + 1) * ISH].T).astype(np.float16)),
        })
    return in_maps


def kernel(hidden_states, gate_w, gate_bias, expert_gate, expert_up,
           expert_down, shared_gate, shared_up, shared_down):
    import os
    # The axon NTFF trace hook is absent in this container; make sure the
    # PJRT execute path never tries to use it.
    os.environ.setdefault("BASS_NEVER_TRACE", "1")
    from concourse.bass_utils import run_bass_kernel_spmd
    nc = _get_module()
    in_maps = make_in_maps(hidden_states, gate_w, gate_bias, expert_gate,
                           expert_up, expert_down, shared_gate, shared_up,
                           shared_down)
    res = run_bass_kernel_spmd(nc, in_maps, core_ids=list(range(N_CORES)))
    out = np.concatenate([np.asarray(res.results[c]["out"], np.float32)
                          for c in range(N_CORES)], axis=0)
    return out.reshape(np.asarray(hidden_states).shape)


# revision 16
# speedup vs baseline: 1.2686x; 1.0011x over previous
# DeepseekV3MoECalibrate Trainium2 kernel (8 NeuronCores, expert-parallel).
#
# Sharding: 32 experts -> 4 per core; shared expert split along the 2I=2048
# intermediate dim (256 rows per core); tokens replicated; partial outputs
# summed with an on-device ReduceScatter.
#
# All weights and the token matrix are pre-transposed AND pre-packed on the
# HOST into the exact [128, free] SBUF layouts the PE needs, fp16, so the
# TensorEngine runs nothing but full-rate fp16 matmuls (no on-device
# transposes, no weight PSUM-evacuation copies) and every weight matrix is
# a single large DMA (per-DMA queue overhead ~0.9us makes small transfers
# expensive).  Router logits are computed exactly from an fp16 hi/lo split
# of x and gate_w (x.gw = xh.gh + xh.gl + xl.gh, error ~1e-7), so top-k
# selection matches the fp32 reference; the rest of the router is fp32 on
# DVE/Act.
#
# Routing weights are applied to the stage-1 activations with a deferred
# in-place scale pass on the Pool engine, so the stage-3 down-projection is
# one 18-matmul PSUM accumulation chain per output tile (4 experts x 4
# i-tiles + 2 shared i-tiles) with a single evacuation.
from contextlib import ExitStack

import numpy as np

import concourse.bass as bass
import concourse.tile as tile
from concourse import bacc, mybir
from concourse.masks import make_identity

F32 = mybir.dt.float32
F32R = mybir.dt.float32r
F16 = mybir.dt.float16
AF = mybir.ActivationFunctionType
OP = mybir.AluOpType
AX = mybir.AxisListType

N_CORES = 8
T, H, I, E = 1024, 1024, 512, 32
E_LOC = E // N_CORES          # 4 experts per core
ISH = 2 * I // N_CORES        # 256 shared-intermediate rows per core
TT = T // 128                 # 8 token tiles
HK = H // 128                 # 8 h k-tiles
IK = I // 128                 # 4 i-tiles per expert
SK = ISH // 128               # 2 shared i-tiles
TH = T // 512                 # 2 t halves (stage-1 rhs width)
NH = H // 512                 # 2 h halves (stage-3 rhs width)

# entry table: (kind, expert idx or None, #i-tiles); shared first so phase A
# can start before the router finishes (no routing weight needed).
ENTRIES = [("shared", None, SK)] + [("expert", e, IK) for e in range(E_LOC)]
N_ITILES = SK + E_LOC * IK    # 18 i-tiles total


def build_module(use_collective=True, num_devices=N_CORES):
    nc = bacc.Bacc("TRN2", target_bir_lowering=False, debug=False,
                   num_devices=num_devices)

    xh_d = nc.dram_tensor("xh", [H, T], F16, kind="ExternalInput")
    xl_d = nc.dram_tensor("xl", [H, T], F16, kind="ExternalInput")
    # gh/gl packed: [128, (ht, {gh,gl}, E)]
    ghl_d = nc.dram_tensor("ghl", [128, HK * 2 * E], F16, kind="ExternalInput")
    bias_d = nc.dram_tensor("bias", [128, E], F32, kind="ExternalInput")
    wselbc_d = nc.dram_tensor("wselbc", [E, E_LOC * 128], F32,
                              kind="ExternalInput")
    # per-expert gate/up packed [128, (ht, I)]; down packed [128, (it, H)]
    wg_d = nc.dram_tensor("wg", [E_LOC, 128, HK * I], F16,
                          kind="ExternalInput")
    wu_d = nc.dram_tensor("wu", [E_LOC, 128, HK * I], F16,
                          kind="ExternalInput")
    wd_d = nc.dram_tensor("wd", [E_LOC, 128, IK * H], F16,
                          kind="ExternalInput")
    sg_d = nc.dram_tensor("sg", [128, HK * ISH], F16, kind="ExternalInput")
    su_d = nc.dram_tensor("su", [128, HK * ISH], F16, kind="ExternalInput")
    sd_d = nc.dram_tensor("sd", [128, SK * H], F16, kind="ExternalInput")
    out_rows = T // num_devices if use_collective else T
    out_d = nc.dram_tensor("out", [out_rows, H], F32, kind="ExternalOutput")

    with tile.TileContext(nc) as tc, ExitStack() as ctx:
        const = ctx.enter_context(tc.tile_pool(name="const", bufs=1))
        sbr = ctx.enter_context(tc.tile_pool(name="router", bufs=2))
        xpool = ctx.enter_context(tc.tile_pool(name="xt", bufs=1))
        xlp = ctx.enter_context(tc.tile_pool(name="xl", bufs=1))
        wgu_pool = ctx.enter_context(tc.tile_pool(name="wgu", bufs=1))
        wd_pool = ctx.enter_context(tc.tile_pool(name="wd", bufs=1))
        a_pool = ctx.enter_context(tc.tile_pool(name="ats", bufs=1))
        wb_pool = ctx.enter_context(tc.tile_pool(name="wb", bufs=1))
        tmp_pool = ctx.enter_context(tc.tile_pool(name="tmp", bufs=3))
        stg_pool = ctx.enter_context(tc.tile_pool(name="stg", bufs=3))
        dram = ctx.enter_context(tc.tile_pool(name="dram", bufs=1, space="DRAM"))

        ps_main = ctx.enter_context(tc.tile_pool(name="ps_main", bufs=4,
                                                 space="PSUM"))
        ps_r = ctx.enter_context(tc.tile_pool(name="ps_r", bufs=3,
                                              space="PSUM"))
        ps_lg = ctx.enter_context(tc.tile_pool(name="ps_lg", bufs=1,
                                               space="PSUM"))

        ident_f = const.tile([128, 128], F32, name="ident_f")
        make_identity(nc, ident_f[:])

        # ---- DMA plan ------------------------------------------------------
        # The DMA engines serve all queues as ONE serial stream (~344 GB/s),
        # so everything goes on the sync/HWDGE queue in exact consumption
        # order: shared g, x tiles (pace the first chains), shared u, gate
        # table, e0 weights, xl stream, router smalls, e1..e3 weights, down
        # weights, outputs.
        sg_sb = wgu_pool.tile([128, HK * ISH], F16, name="sg_sb")
        nc.sync.dma_start(sg_sb[:], sg_d[:])
        xt = [xpool.tile([128, T], F16, name=f"xt{ht}") for ht in range(HK)]
        nc.sync.dma_start(xt[0][:], xh_d[0:128, :])
        su_sb = wgu_pool.tile([128, HK * ISH], F16, name="su_sb")
        nc.sync.dma_start(su_sb[:], su_d[:])
        ghl_sb = sbr.tile([128, HK * 2 * E], F16, name="ghl_sb")
        nc.sync.dma_start(ghl_sb[:], ghl_d[:])
        for ht in range(1, HK):
            nc.sync.dma_start(xt[ht][:], xh_d[ht * 128:(ht + 1) * 128, :])

        wg_sb, wu_sb = [sg_sb], [su_sb]
        for e in range(E_LOC):
            g = wgu_pool.tile([128, HK * I], F16, name=f"wg{e}")
            u = wgu_pool.tile([128, HK * I], F16, name=f"wu{e}")
            if e == 0:
                nc.sync.dma_start(g[:], wg_d[e])
                nc.sync.dma_start(u[:], wu_d[e])
            wg_sb.append(g)
            wu_sb.append(u)

        wd_sb = [wd_pool.tile([128, SK * H], F16, name="sd_sb")]
        for e in range(E_LOC):
            wd_sb.append(wd_pool.tile([128, IK * H], F16, name=f"wd{e}"))

        def late_dmas():
            # issued after the xl stream in queue order
            for e in range(1, E_LOC):
                nc.sync.dma_start(wg_sb[1 + e][:], wg_d[e])
                nc.sync.dma_start(wu_sb[1 + e][:], wu_d[e])
            nc.sync.dma_start(wd_sb[0][:], sd_d[:])
            for e in range(E_LOC):
                nc.sync.dma_start(wd_sb[1 + e][:], wd_d[e])

        bias_bc = sbr.tile([128, E], F32, name="bias_bc")
        wselbc_sb = sbr.tile([E, E_LOC * 128], F32R, name="wselbc_sb")

        a_tiles = [a_pool.tile([128, T], F16, name=f"a{i}")
                   for i in range(N_ITILES)]
        a_base = {}
        off = 0
        for ei, (kind, e, ike) in enumerate(ENTRIES):
            a_base[ei] = off
            off += ike

        # ---- router: exact fp16-split logits -------------------------------
        lgall = ps_lg.tile([128, TT * E], F32, name="lgall")

        def gh_sl(ht):
            return ghl_sb[:, ht * 2 * E:ht * 2 * E + E]

        def gl_sl(ht):
            return ghl_sb[:, ht * 2 * E + E:(ht + 1) * 2 * E]

        def logits12_group(ht):
            # xh.gh + xh.gl terms (no xl dependency).  PSUM start_tensor_calc
            # marks the whole 2KB zero region pending-zero, so ONLY the very
            # first matmul into lgall may set start=True: every slice's first
            # touch then auto-zeroes, later touches accumulate.
            for pi, rh in enumerate((gh_sl(ht), gl_sl(ht))):
                for tt in range(TT):
                    nc.tensor.matmul(
                        lgall[:, tt * E:(tt + 1) * E],
                        xt[ht][:, tt * 128:(tt + 1) * 128],
                        rh,
                        start=(ht == 0 and pi == 0 and tt == 0), stop=False,
                        skip_group_check=True)

        def logits3_group(ht):
            # xl.gh correction term
            xlt = xlp.tile([128, T], F16, name=f"xl{ht}", tag="xl", bufs=5)
            nc.sync.dma_start(xlt[:], xl_d[ht * 128:(ht + 1) * 128, :])
            for tt in range(TT):
                nc.tensor.matmul(
                    lgall[:, tt * E:(tt + 1) * E],
                    xlt[:, tt * 128:(tt + 1) * 128],
                    gh_sl(ht),
                    start=False, stop=(ht == HK - 1),
                    skip_group_check=True)

        # ---- stage 1 for the shared entry: th=0 runs ht-outer across all
        # four PSUM chains so the PE keeps pace with the arriving xt tiles
        # (one matmul per chain per tile) instead of idling on the first
        # chain; logits groups slot in per-ht as extra filler.
        def stage_a0(interleave_ht):
            ike = SK
            ab = a_tiles[0:SK]
            gps = [ps_main.tile([128, 512], F32, name=f"gp0_0_{it}", tag="ps")
                   for it in range(ike)]
            ups = [ps_main.tile([128, 512], F32, name=f"up0_0_{it}", tag="ps")
                   for it in range(ike)]
            for ht in range(HK):
                for it in range(ike):
                    nc.tensor.matmul(
                        gps[it][:],
                        sg_sb[:, (ht * ike + it) * 128:(ht * ike + it + 1) * 128],
                        xt[ht][:, 0:512],
                        start=(ht == 0), stop=(ht == HK - 1))
                    nc.tensor.matmul(
                        ups[it][:],
                        su_sb[:, (ht * ike + it) * 128:(ht * ike + it + 1) * 128],
                        xt[ht][:, 0:512],
                        start=(ht == 0), stop=(ht == HK - 1))
                interleave_ht(ht)
            for it in range(ike):
                sg_t = tmp_pool.tile([128, 512], F32, name=f"sl0_0_{it}",
                                     tag="silu")
                nc.scalar.activation(sg_t[:], gps[it][:], AF.Silu)
                nc.vector.tensor_tensor(ab[it][:, 0:512], sg_t[:], ups[it][:],
                                        OP.mult)
            for it in range(ike):
                gp = ps_main.tile([128, 512], F32, name=f"gp0_1_{it}",
                                  tag="ps")
                up = ps_main.tile([128, 512], F32, name=f"up0_1_{it}",
                                  tag="ps")
                for ht in range(HK):
                    nc.tensor.matmul(
                        gp[:],
                        sg_sb[:, (ht * ike + it) * 128:(ht * ike + it + 1) * 128],
                        xt[ht][:, 512:1024],
                        start=(ht == 0), stop=(ht == HK - 1))
                for ht in range(HK):
                    nc.tensor.matmul(
                        up[:],
                        su_sb[:, (ht * ike + it) * 128:(ht * ike + it + 1) * 128],
                        xt[ht][:, 512:1024],
                        start=(ht == 0), stop=(ht == HK - 1))
                sg_t = tmp_pool.tile([128, 512], F32, name=f"sl0_1_{it}",
                                     tag="silu")
                nc.scalar.activation(sg_t[:], gp[:], AF.Silu)
                nc.vector.tensor_tensor(ab[it][:, 512:1024], sg_t[:], up[:],
                                        OP.mult)

        # ---- stage 1 (gate/up chains + silu*up into aTs) -------------------
        def stage_a(ei, interleave=None):
            kind, e, ike = ENTRIES[ei]
            wgt, wut = wg_sb[ei], wu_sb[ei]
            ab = a_tiles[a_base[ei]:a_base[ei] + ike]
            step = 0
            for th in range(TH):
                for it in range(ike):
                    gp = ps_main.tile([128, 512], F32,
                                      name=f"gp{ei}_{th}_{it}", tag="ps")
                    up = ps_main.tile([128, 512], F32,
                                      name=f"up{ei}_{th}_{it}", tag="ps")
                    for ht in range(HK):
                        nc.tensor.matmul(
                            gp[:],
                            wgt[:, (ht * ike + it) * 128:(ht * ike + it + 1) * 128],
                            xt[ht][:, th * 512:(th + 1) * 512],
                            start=(ht == 0), stop=(ht == HK - 1))
                    for ht in range(HK):
                        nc.tensor.matmul(
                            up[:],
                            wut[:, (ht * ike + it) * 128:(ht * ike + it + 1) * 128],
                            xt[ht][:, th * 512:(th + 1) * 512],
                            start=(ht == 0), stop=(ht == HK - 1))
                    sg_t = tmp_pool.tile([128, 512], F32,
                                         name=f"sl{ei}_{th}_{it}", tag="silu")
                    nc.scalar.activation(sg_t[:], gp[:], AF.Silu)
                    nc.vector.tensor_tensor(
                        ab[it][:, th * 512:(th + 1) * 512],
                        sg_t[:], up[:], OP.mult)
                    if interleave is not None:
                        interleave(step)
                    step += 1

        # ---- router top-k math (DVE/Act only; transposes deferred) --------
        wt_tiles = []

        def routing_math(tt):
            lg = lgall[:, tt * E:(tt + 1) * E]
            S = sbr.tile([128, E], F32, name=f"S{tt}", tag="S")
            nc.scalar.activation(S[:], lg, AF.Sigmoid)
            SC = sbr.tile([128, E], F32, name=f"SC{tt}", tag="SC")
            nc.vector.tensor_tensor(SC[:], S[:], bias_bc[:], OP.add)
            topg = sbr.tile([128, E], F32, name=f"topg{tt}", tag="topg")
            for g in range(4):
                nc.vector.max(topg[:, 8 * g:8 * g + 8], SC[:, 8 * g:8 * g + 8])
            gs8 = sbr.tile([128, 8], F32, name=f"gs8{tt}", tag="gs8")
            nc.vector.memset(gs8[:], -1e30)
            tg = topg[:].rearrange("p (g k) -> p g k", k=8)
            nc.vector.tensor_tensor(gs8[:, 0:4], tg[:, :, 0], tg[:, :, 1],
                                    OP.add)
            gtop = sbr.tile([128, 8], F32, name=f"gtop{tt}", tag="gtop")
            nc.vector.max(gtop[:], gs8[:])
            gmask = sbr.tile([128, 4], F32, name=f"gmask{tt}", tag="gmask")
            nc.vector.tensor_scalar(gmask[:], gs8[:, 0:4], gtop[:, 1:2], None,
                                    OP.is_ge)
            SCm = sbr.tile([128, E], F32, name=f"SCm{tt}", tag="SCm")
            nc.vector.tensor_tensor(
                SCm[:].rearrange("p (g k) -> p g k", k=8),
                SC[:].rearrange("p (g k) -> p g k", k=8),
                gmask[:].rearrange("p (g k) -> p g k", k=1).broadcast_to(
                    [128, 4, 8]),
                OP.mult)
            etop = sbr.tile([128, 8], F32, name=f"etop{tt}", tag="etop")
            nc.vector.max(etop[:], SCm[:])
            sel = sbr.tile([128, E], F32, name=f"sel{tt}", tag="sel")
            nc.vector.tensor_scalar(sel[:], SCm[:], etop[:, 7:8], None,
                                    OP.is_ge)
            wr = sbr.tile([128, E], F32, name=f"wr{tt}", tag="wr")
            nc.vector.tensor_tensor(wr[:], S[:], sel[:], OP.mult)
            den = sbr.tile([128, 1], F32, name=f"den{tt}", tag="den")
            nc.vector.reduce_sum(den[:], wr[:], axis=AX.X)
            nc.vector.tensor_scalar(den[:], den[:], 1.0 / 2.5, None, OP.mult)
            dinv = sbr.tile([128, 1], F32, name=f"dinv{tt}", tag="dinv")
            nc.vector.reciprocal(dinv[:], den[:])
            wt = sbr.tile([128, E], F32, name=f"wt{tt}", tag="wt", bufs=8)
            nc.vector.tensor_scalar(wt[:], wr[:], dinv[:], None, OP.mult)
            wt_tiles.append(wt)

        # ================= emission schedule ===============================
        # Shared entry first (needs no routing weights); logit terms with no
        # xl dependency slot into its 4 stage-1 steps, the xl correction term
        # into expert-0's first steps.  Router math then lands between e0 and
        # e1 stage-2 work on DVE/Act; wt transposes + wb rows run on the PE
        # right after (DVE router is done by then), so expert-1 stage-2 never
        # queues behind them.
        stage_a0(interleave_ht=logits12_group)
        stage_a(1, interleave=lambda s: (logits3_group(2 * s),
                                         logits3_group(2 * s + 1))
                if s < 4 else None)
        nc.sync.dma_start(bias_bc[:], bias_d[:])
        nc.sync.dma_start(wselbc_sb[:], wselbc_d[:].bitcast(F32R))
        late_dmas()
        for tt in range(TT):
            routing_math(tt)

        # wt transposes + routing-weight broadcast rows
        wT_r = sbr.tile([E, T], F32R, name="wT_r")
        for tt in range(TT):
            p = ps_r.tile([128, 512], F32, name=f"wtp{tt}", tag="ps_r")
            nc.tensor.transpose(p[0:E, 0:128], wt_tiles[tt][:], ident_f[:])
            nc.vector.tensor_copy(wT_r[:, tt * 128:(tt + 1) * 128].bitcast(F32R),
                                  p[0:E, 0:128].bitcast(F32R))
        wb_tiles = []
        for e in range(E_LOC):
            wbt = wb_pool.tile([128, T], F16, name=f"wb{e}")
            for th in range(TH):
                p = ps_r.tile([128, 512], F32, name=f"wbp{e}_{th}", tag="ps_r")
                nc.tensor.matmul(p[:], wselbc_sb[:, e * 128:(e + 1) * 128],
                                 wT_r[:, th * 512:(th + 1) * 512],
                                 start=True, stop=True)
                nc.vector.tensor_copy(wbt[:, th * 512:(th + 1) * 512], p[:])
            wb_tiles.append(wbt)

        # deferred routing-weight scale: in-place on the Pool engine.
        def scale_pass(ei):
            kind, e, ike = ENTRIES[ei]
            ab = a_tiles[a_base[ei]:a_base[ei] + ike]
            for th in range(TH):
                for it in range(ike):
                    sl = ab[it][:, th * 512:(th + 1) * 512]
                    nc.gpsimd.tensor_tensor(
                        sl, sl, wb_tiles[e][:, th * 512:(th + 1) * 512],
                        OP.mult)

        scale_pass(1)
        stage_a(2)
        scale_pass(2)
        for ei in range(3, len(ENTRIES)):
            stage_a(ei)
            scale_pass(ei)

        # ---- stage 3: one 18-matmul PSUM chain per output tile ------------
        if use_collective:
            bin_t = dram.tile([T, H], F32, name="rsin")
            target = bin_t
        else:
            target = out_d
        for tt in range(TT):
            for hh in range(NH):
                op = ps_main.tile([128, 512], F32, name=f"o{tt}_{hh}",
                                  tag="ps")
                n = 0
                for ei, (kind, e, ike) in enumerate(ENTRIES):
                    for it in range(ike):
                        nc.tensor.matmul(
                            op[:],
                            a_tiles[a_base[ei] + it][:, tt * 128:(tt + 1) * 128],
                            wd_sb[ei][:, it * H + hh * 512:
                                      it * H + (hh + 1) * 512],
                            start=(n == 0), stop=(n == N_ITILES - 1))
                        n += 1
                st = stg_pool.tile([128, 512], F32, name=f"st{tt}_{hh}",
                                   tag="stg")
                nc.vector.tensor_copy(st[:], op[:])
                nc.sync.dma_start(
                    target[tt * 128:(tt + 1) * 128, hh * 512:(hh + 1) * 512],
                    st[:])

        # ---- ReduceScatter + output ---------------------------------------
        if use_collective:
            bout_t = dram.tile([out_rows, H], F32, name="rsout")
            nc.gpsimd.collective_compute(
                "ReduceScatter", OP.add,
                replica_groups=[list(range(num_devices))],
                ins=[bin_t.opt()], outs=[bout_t.opt()])
            nc.sync.dma_start(out_d[:], bout_t[:])
    nc.compile()
    return nc


_NC_CACHE = {}


def _get_module():
    key = "spmd"
    if key not in _NC_CACHE:
        _NC_CACHE[key] = build_module(use_collective=True, num_devices=N_CORES)
    return _NC_CACHE[key]


def _pack_rows(a, blk=128):
    """[R, C] -> [128, (R//128) * C]: row-tile r128 layout for one-DMA loads."""
    r, c = a.shape
    return np.ascontiguousarray(
        a.reshape(r // blk, blk, c).transpose(1, 0, 2).reshape(blk, -1))


def make_in_maps(hidden_states, gate_w, gate_bias, expert_gate, expert_up,
                 expert_down, shared_gate, shared_up, shared_down):
    x = np.asarray(hidden_states, np.float32).reshape(T, H)
    xt = np.ascontiguousarray(x.T)                       # [H, T]
    xh = xt.astype(np.float16)
    xl = (xt - xh.astype(np.float32)).astype(np.float16)
    gwt = np.ascontiguousarray(np.asarray(gate_w, np.float32).T)  # [H, E]
    gh = gwt.astype(np.float16)
    gl = (gwt - gh.astype(np.float32)).astype(np.float16)
    # pack gh/gl as [128, (ht, {gh,gl}, E)]
    ghl = np.concatenate(
        [gh.reshape(HK, 128, E)[:, :, None, :],
         gl.reshape(HK, 128, E)[:, :, None, :]], axis=2)  # [HK,128,2,E]
    ghl = np.ascontiguousarray(
        ghl.transpose(1, 0, 2, 3).reshape(128, HK * 2 * E))
    bias = np.broadcast_to(
        np.asarray(gate_bias, np.float32).reshape(1, E), (128, E))
    bias = np.ascontiguousarray(bias)
    eg = np.asarray(expert_gate, np.float32)
    eu = np.asarray(expert_up, np.float32)
    ed = np.asarray(expert_down, np.float32)
    sgT = np.asarray(shared_gate, np.float32).T          # [H, 2I]
    suT = np.asarray(shared_up, np.float32).T            # [H, 2I]
    sd = np.asarray(shared_down, np.float32)             # [H, 2I]
    in_maps = []
    for c in range(N_CORES):
        lo, hi = c * E_LOC, (c + 1) * E_LOC
        wselbc = np.zeros((E, E_LOC * 128), np.float32)
        for j in range(E_LOC):
            wselbc[lo + j, j * 128:(j + 1) * 128] = 1.0
        wg = np.stack([_pack_rows(eg[lo + j].T.astype(np.float16))
                       for j in range(E_LOC)])           # [E_LOC,128,HK*I]
        wu = np.stack([_pack_rows(eu[lo + j].T.astype(np.float16))
                       for j in range(E_LOC)])
        wd = np.stack([_pack_rows(ed[lo + j].T.astype(np.float16))
                       for j in range(E_LOC)])           # [E_LOC,128,IK*H]
        in_maps.append({
            "xh": xh, "xl": xl, "ghl": ghl, "bias": bias, "wselbc": wselbc,
            "wg": wg, "wu": wu, "wd": wd,
            "sg": _pack_rows(np.ascontiguousarray(
                sgT[:, c * ISH:(c + 1) * ISH]).astype(np.float16)),
            "su": _pack_rows(np.ascontiguousarray(
                suT[:, c * ISH:(c + 1) * ISH]).astype(np.float16)),
            "sd": _pack_rows(np.ascontiguousarray(
                sd[:, c * ISH:(c + 1) * ISH].T).astype(np.float16)),
        })
    return in_maps


def kernel(hidden_states, gate_w, gate_bias, expert_gate, expert_up,
           expert_down, shared_gate, shared_up, shared_down):
    import os
    # The axon NTFF trace hook is absent in this container; make sure the
    # PJRT execute path never tries to use it.
    os.environ.setdefault("BASS_NEVER_TRACE", "1")
    from concourse.bass_utils import run_bass_kernel_spmd
    nc = _get_module()
    in_maps = make_in_maps(hidden_states, gate_w, gate_bias, expert_gate,
                           expert_up, expert_down, shared_gate, shared_up,
                           shared_down)
    res = run_bass_kernel_spmd(nc, in_maps, core_ids=list(range(N_CORES)))
    out = np.concatenate([np.asarray(res.results[c]["out"], np.float32)
                          for c in range(N_CORES)], axis=0)
    return out.reshape(np.asarray(hidden_states).shape)


# revision 20
# speedup vs baseline: 1.5359x; 1.2107x over previous
# DeepseekV3MoECalibrate Trainium2 kernel (8 NeuronCores, expert-parallel).
#
# Sharding: 32 experts -> 4 per core; shared expert split along the 2I=2048
# intermediate dim (256 rows per core); tokens replicated; partial outputs
# summed with an on-device ReduceScatter.
#
# All weights and the token matrix are pre-transposed AND pre-packed on the
# HOST into the exact [128, free] SBUF layouts the PE needs, fp16, so the
# TensorEngine runs nothing but full-rate fp16 matmuls (no on-device
# transposes, no weight PSUM-evacuation copies) and every weight matrix is
# a single large DMA (per-DMA queue overhead ~0.9us makes small transfers
# expensive).  Router logits are computed exactly from an fp16 hi/lo split
# of x and gate_w (x.gw = xh.gh + xh.gl + xl.gh, error ~1e-7), so top-k
# selection matches the fp32 reference; the rest of the router is fp32 on
# DVE/Act.
#
# Routing weights are applied to the stage-1 activations with a deferred
# in-place scale pass on the Pool engine, so the stage-3 down-projection is
# one 18-matmul PSUM accumulation chain per output tile (4 experts x 4
# i-tiles + 2 shared i-tiles) with a single evacuation.
from contextlib import ExitStack

import numpy as np

import concourse.bass as bass
import concourse.tile as tile
from concourse import bacc, mybir
from concourse.masks import make_identity

F32 = mybir.dt.float32
F32R = mybir.dt.float32r
F16 = mybir.dt.float16
AF = mybir.ActivationFunctionType
OP = mybir.AluOpType
AX = mybir.AxisListType

N_CORES = 8
T, H, I, E = 1024, 1024, 512, 32
E_LOC = E // N_CORES          # 4 experts per core
ISH = 2 * I // N_CORES        # 256 shared-intermediate rows per core
TT = T // 128                 # 8 token tiles
HK = H // 128                 # 8 h k-tiles
IK = I // 128                 # 4 i-tiles per expert
SK = ISH // 128               # 2 shared i-tiles
TH = T // 512                 # 2 t halves (stage-1 rhs width)
NH = H // 512                 # 2 h halves (stage-3 rhs width)

# entry table: (kind, expert idx or None, #i-tiles); shared first so phase A
# can start before the router finishes (no routing weight needed).
ENTRIES = [("shared", None, SK)] + [("expert", e, IK) for e in range(E_LOC)]
N_ITILES = SK + E_LOC * IK    # 18 i-tiles total


def build_module(use_collective=True, num_devices=N_CORES):
    nc = bacc.Bacc("TRN2", target_bir_lowering=False, debug=False,
                   num_devices=num_devices)

    xh_d = nc.dram_tensor("xh", [H, T], F16, kind="ExternalInput")
    xl_d = nc.dram_tensor("xl", [H, T], F16, kind="ExternalInput")
    # gh/gl packed: [128, (ht, {gh,gl}, E)]
    ghl_d = nc.dram_tensor("ghl", [128, HK * 2 * E], F16, kind="ExternalInput")
    bias_d = nc.dram_tensor("bias", [128, E], F32, kind="ExternalInput")
    wselbc_d = nc.dram_tensor("wselbc", [E, E_LOC * 128], F32,
                              kind="ExternalInput")
    # per-expert gate/up packed [128, (ht, I)]; down packed [128, (it, H)]
    wg_d = nc.dram_tensor("wg", [E_LOC, 128, HK * I], F16,
                          kind="ExternalInput")
    wu_d = nc.dram_tensor("wu", [E_LOC, 128, HK * I], F16,
                          kind="ExternalInput")
    wd_d = nc.dram_tensor("wd", [E_LOC, 128, IK * H], F16,
                          kind="ExternalInput")
    sg_d = nc.dram_tensor("sg", [128, HK * ISH], F16, kind="ExternalInput")
    su_d = nc.dram_tensor("su", [128, HK * ISH], F16, kind="ExternalInput")
    sd_d = nc.dram_tensor("sd", [128, SK * H], F16, kind="ExternalInput")
    out_rows = T // num_devices if use_collective else T
    out_d = nc.dram_tensor("out", [out_rows, H], F32, kind="ExternalOutput")

    with tile.TileContext(nc) as tc, ExitStack() as ctx:
        const = ctx.enter_context(tc.tile_pool(name="const", bufs=1))
        sbr = ctx.enter_context(tc.tile_pool(name="router", bufs=2))
        xpool = ctx.enter_context(tc.tile_pool(name="xt", bufs=1))
        xlp = ctx.enter_context(tc.tile_pool(name="xl", bufs=1))
        wgu_pool = ctx.enter_context(tc.tile_pool(name="wgu", bufs=1))
        wd_pool = ctx.enter_context(tc.tile_pool(name="wd", bufs=1))
        a_pool = ctx.enter_context(tc.tile_pool(name="ats", bufs=1))
        wb_pool = ctx.enter_context(tc.tile_pool(name="wb", bufs=1))
        tmp_pool = ctx.enter_context(tc.tile_pool(name="tmp", bufs=3))
        stg_pool = ctx.enter_context(tc.tile_pool(name="stg", bufs=3))
        dram = ctx.enter_context(tc.tile_pool(name="dram", bufs=1, space="DRAM"))

        ps_main = ctx.enter_context(tc.tile_pool(name="ps_main", bufs=4,
                                                 space="PSUM"))
        ps_r = ctx.enter_context(tc.tile_pool(name="ps_r", bufs=3,
                                              space="PSUM"))
        ps_lg = ctx.enter_context(tc.tile_pool(name="ps_lg", bufs=1,
                                               space="PSUM"))

        ident_f = const.tile([128, 128], F32, name="ident_f")
        make_identity(nc, ident_f[:])

        # ---- DMA plan ------------------------------------------------------
        # The DMA engines serve all queues as ONE serial stream (~344 GB/s),
        # so everything goes on the sync/HWDGE queue in exact consumption
        # order: shared g, x tiles (pace the first chains), shared u, gate
        # table, e0 weights, xl stream, router smalls, e1..e3 weights, down
        # weights, outputs.
        sg_sb = wgu_pool.tile([128, HK * ISH], F16, name="sg_sb")
        nc.sync.dma_start(sg_sb[:], sg_d[:])
        xt = [xpool.tile([128, T], F16, name=f"xt{ht}") for ht in range(HK)]
        nc.sync.dma_start(xt[0][:], xh_d[0:128, :])
        su_sb = wgu_pool.tile([128, HK * ISH], F16, name="su_sb")
        nc.sync.dma_start(su_sb[:], su_d[:])
        ghl_sb = sbr.tile([128, HK * 2 * E], F16, name="ghl_sb")
        nc.sync.dma_start(ghl_sb[:], ghl_d[:])
        for ht in range(1, HK):
            nc.sync.dma_start(xt[ht][:], xh_d[ht * 128:(ht + 1) * 128, :])

        wg_sb, wu_sb = [sg_sb], [su_sb]
        for e in range(E_LOC):
            g = wgu_pool.tile([128, HK * I], F16, name=f"wg{e}")
            u = wgu_pool.tile([128, HK * I], F16, name=f"wu{e}")
            if e == 0:
                nc.sync.dma_start(g[:], wg_d[e])
                nc.sync.dma_start(u[:], wu_d[e])
            wg_sb.append(g)
            wu_sb.append(u)

        wd_sb = [wd_pool.tile([128, SK * H], F16, name="sd_sb")]
        for e in range(E_LOC):
            wd_sb.append(wd_pool.tile([128, IK * H], F16, name=f"wd{e}"))

        def late_dmas():
            # issued after the xl stream in queue order
            for e in range(1, E_LOC):
                nc.sync.dma_start(wg_sb[1 + e][:], wg_d[e])
                nc.sync.dma_start(wu_sb[1 + e][:], wu_d[e])
            nc.sync.dma_start(wd_sb[0][:], sd_d[:])
            for e in range(E_LOC):
                nc.sync.dma_start(wd_sb[1 + e][:], wd_d[e])

        bias_bc = sbr.tile([128, E], F32, name="bias_bc")
        wselbc_sb = sbr.tile([E, E_LOC * 128], F32R, name="wselbc_sb")

        a_tiles = [a_pool.tile([128, T], F16, name=f"a{i}")
                   for i in range(N_ITILES)]
        a_base = {}
        off = 0
        for ei, (kind, e, ike) in enumerate(ENTRIES):
            a_base[ei] = off
            off += ike

        # ---- router: exact fp16-split logits -------------------------------
        lgall = ps_lg.tile([128, TT * E], F32, name="lgall")

        def gh_sl(ht):
            return ghl_sb[:, ht * 2 * E:ht * 2 * E + E]

        def gl_sl(ht):
            return ghl_sb[:, ht * 2 * E + E:(ht + 1) * 2 * E]

        def logits12_group(ht):
            # xh.gh + xh.gl terms (no xl dependency).  PSUM start_tensor_calc
            # marks the whole 2KB zero region pending-zero, so ONLY the very
            # first matmul into lgall may set start=True: every slice's first
            # touch then auto-zeroes, later touches accumulate.
            for pi, rh in enumerate((gh_sl(ht), gl_sl(ht))):
                for tt in range(TT):
                    nc.tensor.matmul(
                        lgall[:, tt * E:(tt + 1) * E],
                        xt[ht][:, tt * 128:(tt + 1) * 128],
                        rh,
                        start=(ht == 0 and pi == 0 and tt == 0), stop=False,
                        skip_group_check=True)

        def logits3_group(ht):
            # xl.gh correction term
            xlt = xlp.tile([128, T], F16, name=f"xl{ht}", tag="xl", bufs=5)
            nc.sync.dma_start(xlt[:], xl_d[ht * 128:(ht + 1) * 128, :])
            for tt in range(TT):
                nc.tensor.matmul(
                    lgall[:, tt * E:(tt + 1) * E],
                    xlt[:, tt * 128:(tt + 1) * 128],
                    gh_sl(ht),
                    start=False, stop=(ht == HK - 1),
                    skip_group_check=True)

        # ---- stage 1 for the shared entry: th=0 runs ht-outer across all
        # four PSUM chains so the PE keeps pace with the arriving xt tiles
        # (one matmul per chain per tile) instead of idling on the first
        # chain; logits groups slot in per-ht as extra filler.
        def stage_a0(interleave_ht):
            ike = SK
            ab = a_tiles[0:SK]
            gps = [ps_main.tile([128, 512], F32, name=f"gp0_0_{it}", tag="ps")
                   for it in range(ike)]
            ups = [ps_main.tile([128, 512], F32, name=f"up0_0_{it}", tag="ps")
                   for it in range(ike)]
            for ht in range(HK):
                for it in range(ike):
                    nc.tensor.matmul(
                        gps[it][:],
                        sg_sb[:, (ht * ike + it) * 128:(ht * ike + it + 1) * 128],
                        xt[ht][:, 0:512],
                        start=(ht == 0), stop=(ht == HK - 1))
                    nc.tensor.matmul(
                        ups[it][:],
                        su_sb[:, (ht * ike + it) * 128:(ht * ike + it + 1) * 128],
                        xt[ht][:, 0:512],
                        start=(ht == 0), stop=(ht == HK - 1))
                interleave_ht(ht)
            for it in range(ike):
                sg_t = tmp_pool.tile([128, 512], F32, name=f"sl0_0_{it}",
                                     tag="silu")
                nc.scalar.activation(sg_t[:], gps[it][:], AF.Silu)
                nc.vector.tensor_tensor(ab[it][:, 0:512], sg_t[:], ups[it][:],
                                        OP.mult)
            for it in range(ike):
                gp = ps_main.tile([128, 512], F32, name=f"gp0_1_{it}",
                                  tag="ps")
                up = ps_main.tile([128, 512], F32, name=f"up0_1_{it}",
                                  tag="ps")
                for ht in range(HK):
                    nc.tensor.matmul(
                        gp[:],
                        sg_sb[:, (ht * ike + it) * 128:(ht * ike + it + 1) * 128],
                        xt[ht][:, 512:1024],
                        start=(ht == 0), stop=(ht == HK - 1))
                for ht in range(HK):
                    nc.tensor.matmul(
                        up[:],
                        su_sb[:, (ht * ike + it) * 128:(ht * ike + it + 1) * 128],
                        xt[ht][:, 512:1024],
                        start=(ht == 0), stop=(ht == HK - 1))
                sg_t = tmp_pool.tile([128, 512], F32, name=f"sl0_1_{it}",
                                     tag="silu")
                nc.scalar.activation(sg_t[:], gp[:], AF.Silu)
                nc.vector.tensor_tensor(ab[it][:, 512:1024], sg_t[:], up[:],
                                        OP.mult)

        # ---- stage 1 (gate/up chains + silu*up into aTs) -------------------
        def stage_a(ei, interleave=None):
            kind, e, ike = ENTRIES[ei]
            wgt, wut = wg_sb[ei], wu_sb[ei]
            ab = a_tiles[a_base[ei]:a_base[ei] + ike]
            step = 0
            for th in range(TH):
                for it in range(ike):
                    gp = ps_main.tile([128, 512], F32,
                                      name=f"gp{ei}_{th}_{it}", tag="ps")
                    up = ps_main.tile([128, 512], F32,
                                      name=f"up{ei}_{th}_{it}", tag="ps")
                    for ht in range(HK):
                        nc.tensor.matmul(
                            gp[:],
                            wgt[:, (ht * ike + it) * 128:(ht * ike + it + 1) * 128],
                            xt[ht][:, th * 512:(th + 1) * 512],
                            start=(ht == 0), stop=(ht == HK - 1))
                    for ht in range(HK):
                        nc.tensor.matmul(
                            up[:],
                            wut[:, (ht * ike + it) * 128:(ht * ike + it + 1) * 128],
                            xt[ht][:, th * 512:(th + 1) * 512],
                            start=(ht == 0), stop=(ht == HK - 1))
                    sg_t = tmp_pool.tile([128, 512], F32,
                                         name=f"sl{ei}_{th}_{it}", tag="silu")
                    nc.scalar.activation(sg_t[:], gp[:], AF.Silu)
                    nc.vector.tensor_tensor(
                        ab[it][:, th * 512:(th + 1) * 512],
                        sg_t[:], up[:], OP.mult)
                    if interleave is not None:
                        interleave(step)
                    step += 1

        # ---- router top-k math (DVE/Act only; transposes deferred) --------
        wt_tiles = []

        def routing_math(tt):
            lg = lgall[:, tt * E:(tt + 1) * E]
            S = sbr.tile([128, E], F32, name=f"S{tt}", tag="S")
            nc.scalar.activation(S[:], lg, AF.Sigmoid)
            SC = sbr.tile([128, E], F32, name=f"SC{tt}", tag="SC")
            nc.vector.tensor_tensor(SC[:], S[:], bias_bc[:], OP.add)
            topg = sbr.tile([128, E], F32, name=f"topg{tt}", tag="topg")
            for g in range(4):
                nc.vector.max(topg[:, 8 * g:8 * g + 8], SC[:, 8 * g:8 * g + 8])
            gs8 = sbr.tile([128, 8], F32, name=f"gs8{tt}", tag="gs8")
            nc.vector.memset(gs8[:], -1e30)
            tg = topg[:].rearrange("p (g k) -> p g k", k=8)
            nc.vector.tensor_tensor(gs8[:, 0:4], tg[:, :, 0], tg[:, :, 1],
                                    OP.add)
            gtop = sbr.tile([128, 8], F32, name=f"gtop{tt}", tag="gtop")
            nc.vector.max(gtop[:], gs8[:])
            gmask = sbr.tile([128, 4], F32, name=f"gmask{tt}", tag="gmask")
            nc.vector.tensor_scalar(gmask[:], gs8[:, 0:4], gtop[:, 1:2], None,
                                    OP.is_ge)
            SCm = sbr.tile([128, E], F32, name=f"SCm{tt}", tag="SCm")
            nc.vector.tensor_tensor(
                SCm[:].rearrange("p (g k) -> p g k", k=8),
                SC[:].rearrange("p (g k) -> p g k", k=8),
                gmask[:].rearrange("p (g k) -> p g k", k=1).broadcast_to(
                    [128, 4, 8]),
                OP.mult)
            etop = sbr.tile([128, 8], F32, name=f"etop{tt}", tag="etop")
            nc.vector.max(etop[:], SCm[:])
            sel = sbr.tile([128, E], F32, name=f"sel{tt}", tag="sel")
            nc.vector.tensor_scalar(sel[:], SCm[:], etop[:, 7:8], None,
                                    OP.is_ge)
            wr = sbr.tile([128, E], F32, name=f"wr{tt}", tag="wr")
            nc.vector.tensor_tensor(wr[:], S[:], sel[:], OP.mult)
            den = sbr.tile([128, 1], F32, name=f"den{tt}", tag="den")
            nc.vector.reduce_sum(den[:], wr[:], axis=AX.X)
            nc.vector.tensor_scalar(den[:], den[:], 1.0 / 2.5, None, OP.mult)
            dinv = sbr.tile([128, 1], F32, name=f"dinv{tt}", tag="dinv")
            nc.vector.reciprocal(dinv[:], den[:])
            wt = sbr.tile([128, E], F32, name=f"wt{tt}", tag="wt", bufs=8)
            nc.vector.tensor_scalar(wt[:], wr[:], dinv[:], None, OP.mult)
            wt_tiles.append(wt)

        # ================= emission schedule ===============================
        # Shared entry first (needs no routing weights); logit terms with no
        # xl dependency slot into its 4 stage-1 steps, the xl correction term
        # into expert-0's first steps.  Router math then lands between e0 and
        # e1 stage-2 work on DVE/Act; wt transposes + wb rows run on the PE
        # right after (DVE router is done by then), so expert-1 stage-2 never
        # queues behind them.
        stage_a0(interleave_ht=logits12_group)
        stage_a(1, interleave=lambda s: (logits3_group(2 * s),
                                         logits3_group(2 * s + 1))
                if s < 4 else None)
        nc.sync.dma_start(bias_bc[:], bias_d[:])
        nc.sync.dma_start(wselbc_sb[:], wselbc_d[:].bitcast(F32R))
        late_dmas()
        for tt in range(TT):
            routing_math(tt)

        # wt transposes + routing-weight broadcast rows
        wT_r = sbr.tile([E, T], F32R, name="wT_r")
        for tt in range(TT):
            p = ps_r.tile([128, 512], F32, name=f"wtp{tt}", tag="ps_r")
            nc.tensor.transpose(p[0:E, 0:128], wt_tiles[tt][:], ident_f[:])
            nc.vector.tensor_copy(wT_r[:, tt * 128:(tt + 1) * 128].bitcast(F32R),
                                  p[0:E, 0:128].bitcast(F32R))
        wb_tiles = []
        for e in range(E_LOC):
            wbt = wb_pool.tile([128, T], F16, name=f"wb{e}")
            for th in range(TH):
                p = ps_r.tile([128, 512], F32, name=f"wbp{e}_{th}", tag="ps_r")
                nc.tensor.matmul(p[:], wselbc_sb[:, e * 128:(e + 1) * 128],
                                 wT_r[:, th * 512:(th + 1) * 512],
                                 start=True, stop=True)
                nc.vector.tensor_copy(wbt[:, th * 512:(th + 1) * 512], p[:])
            wb_tiles.append(wbt)

        # deferred routing-weight scale: in-place on the Pool engine.
        def scale_pass(ei):
            kind, e, ike = ENTRIES[ei]
            ab = a_tiles[a_base[ei]:a_base[ei] + ike]
            for th in range(TH):
                for it in range(ike):
                    sl = ab[it][:, th * 512:(th + 1) * 512]
                    nc.gpsimd.tensor_tensor(
                        sl, sl, wb_tiles[e][:, th * 512:(th + 1) * 512],
                        OP.mult)

        scale_pass(1)
        stage_a(2)
        scale_pass(2)
        for ei in range(3, len(ENTRIES)):
            stage_a(ei)
            scale_pass(ei)

        # ---- stage 3: one 18-matmul PSUM chain per output tile ------------
        if use_collective:
            bin_t = dram.tile([T, H], F32, name="rsin")
            target = bin_t
        else:
            target = out_d
        for tt in range(TT):
            for hh in range(NH):
                op = ps_main.tile([128, 512], F32, name=f"o{tt}_{hh}",
                                  tag="ps")
                n = 0
                for ei, (kind, e, ike) in enumerate(ENTRIES):
                    for it in range(ike):
                        nc.tensor.matmul(
                            op[:],
                            a_tiles[a_base[ei] + it][:, tt * 128:(tt + 1) * 128],
                            wd_sb[ei][:, it * H + hh * 512:
                                      it * H + (hh + 1) * 512],
                            start=(n == 0), stop=(n == N_ITILES - 1))
                        n += 1
                st = stg_pool.tile([128, 512], F32, name=f"st{tt}_{hh}",
                                   tag="stg")
                nc.vector.tensor_copy(st[:], op[:])
                nc.sync.dma_start(
                    target[tt * 128:(tt + 1) * 128, hh * 512:(hh + 1) * 512],
                    st[:])

        # ---- ReduceScatter + output ---------------------------------------
        if use_collective:
            bout_t = dram.tile([out_rows, H], F32, name="rsout")
            nc.gpsimd.collective_compute(
                "ReduceScatter", OP.add,
                replica_groups=[list(range(num_devices))],
                ins=[bin_t.opt()], outs=[bout_t.opt()])
            nc.sync.dma_start(out_d[:], bout_t[:])
    nc.compile()
    return nc


_NC_CACHE = {}


def _get_module():
    key = "spmd"
    if key not in _NC_CACHE:
        _NC_CACHE[key] = build_module(use_collective=True, num_devices=N_CORES)
    return _NC_CACHE[key]


def _pack_rows(a, blk=128):
    """[R, C] -> [128, (R//128) * C]: row-tile r128 layout for one-DMA loads."""
    r, c = a.shape
    return np.ascontiguousarray(
        a.reshape(r // blk, blk, c).transpose(1, 0, 2).reshape(blk, -1))


def make_in_maps(hidden_states, gate_w, gate_bias, expert_gate, expert_up,
                 expert_down, shared_gate, shared_up, shared_down):
    x = np.asarray(hidden_states, np.float32).reshape(T, H)
    xt = np.ascontiguousarray(x.T)                       # [H, T]
    xh = xt.astype(np.float16)
    xl = (xt - xh.astype(np.float32)).astype(np.float16)
    gwt = np.ascontiguousarray(np.asarray(gate_w, np.float32).T)  # [H, E]
    gh = gwt.astype(np.float16)
    gl = (gwt - gh.astype(np.float32)).astype(np.float16)
    # pack gh/gl as [128, (ht, {gh,gl}, E)]
    ghl = np.concatenate(
        [gh.reshape(HK, 128, E)[:, :, None, :],
         gl.reshape(HK, 128, E)[:, :, None, :]], axis=2)  # [HK,128,2,E]
    ghl = np.ascontiguousarray(
        ghl.transpose(1, 0, 2, 3).reshape(128, HK * 2 * E))
    bias = np.broadcast_to(
        np.asarray(gate_bias, np.float32).reshape(1, E), (128, E))
    bias = np.ascontiguousarray(bias)
    eg = np.asarray(expert_gate, np.float32)
    eu = np.asarray(expert_up, np.float32)
    ed = np.asarray(expert_down, np.float32)
    sgT = np.asarray(shared_gate, np.float32).T          # [H, 2I]
    suT = np.asarray(shared_up, np.float32).T            # [H, 2I]
    sd = np.asarray(shared_down, np.float32)             # [H, 2I]
    in_maps = []
    for c in range(N_CORES):
        lo, hi = c * E_LOC, (c + 1) * E_LOC
        wselbc = np.zeros((E, E_LOC * 128), np.float32)
        for j in range(E_LOC):
            wselbc[lo + j, j * 128:(j + 1) * 128] = 1.0
        wg = np.stack([_pack_rows(eg[lo + j].T.astype(np.float16))
                       for j in range(E_LOC)])           # [E_LOC,128,HK*I]
        wu = np.stack([_pack_rows(eu[lo + j].T.astype(np.float16))
                       for j in range(E_LOC)])
        wd = np.stack([_pack_rows(ed[lo + j].T.astype(np.float16))
                       for j in range(E_LOC)])           # [E_LOC,128,IK*H]
        in_maps.append({
            "xh": xh, "xl": xl, "ghl": ghl, "bias": bias, "wselbc": wselbc,
            "wg": wg, "wu": wu, "wd": wd,
            "sg": _pack_rows(np.ascontiguousarray(
                sgT[:, c * ISH:(c + 1) * ISH]).astype(np.float16)),
            "su": _pack_rows(np.ascontiguousarray(
                suT[:, c * ISH:(c + 1) * ISH]).astype(np.float16)),
            "sd": _pack_rows(np.ascontiguousarray(
                sd[:, c * ISH:(c + 1) * ISH].T).astype(np.float16)),
        })
    return in_maps


def kernel(hidden_states, gate_w, gate_bias, expert_gate, expert_up,
           expert_down, shared_gate, shared_up, shared_down):
    import os
    # The axon NTFF trace hook is absent in this container; make sure the
    # PJRT execute path never tries to use it.
    os.environ.setdefault("BASS_NEVER_TRACE", "1")
    from concourse.bass_utils import run_bass_kernel_spmd
    nc = _get_module()
    in_maps = make_in_maps(hidden_states, gate_w, gate_bias, expert_gate,
                           expert_up, expert_down, shared_gate, shared_up,
                           shared_down)
    res = run_bass_kernel_spmd(nc, in_maps, core_ids=list(range(N_CORES)))
    out = np.concatenate([np.asarray(res.results[c]["out"], np.float32)
                          for c in range(N_CORES)], axis=0)
    return out.reshape(np.asarray(hidden_states).shape)


# revision 26
# speedup vs baseline: 1.7445x; 1.1358x over previous
# DeepseekV3MoECalibrate Trainium2 kernel (8 NeuronCores, expert-parallel).
#
# Sharding: 32 experts -> 4 per core; shared expert split along the 2I=2048
# intermediate dim (256 rows per core); tokens replicated; partial outputs
# summed with an on-device ReduceScatter.
#
# All weights and the token matrix are pre-transposed AND pre-packed on the
# HOST into the exact [128, free] SBUF layouts the PE needs, so the
# TensorEngine runs nothing but full-rate matmuls (no on-device transposes,
# no weight PSUM-evacuation copies) and every weight matrix is a single
# large DMA (per-DMA queue overhead ~0.9us makes small transfers expensive).
#
# Stage-1 (gate/up projections) runs in fp8e4 DoubleRow perf mode (K=256
# per instruction, 0.5 cycles/row) using a hi/lo fp8 split of both operands:
#   W.X ~= Wh.Xh + Wl.Xh + Wh.Xl   (error ~0.2-0.4%, vs the 2e-2 gate)
# Operands are pre-scaled by powers of two on the host (x*4, w*256) to
# avoid the fp8 denormal range; the 1/1024 descale is applied exactly via
# the silu's input scale and folded out of the up-path at the stage-3
# PSUM evacuation (tensor_scalar instead of tensor_copy, same cost).
# Stage-3 (down projection) stays fp16: one 18-matmul PSUM accumulation
# chain per output tile (4 experts x 4 i-tiles + 2 shared i-tiles).
#
# Router logits are computed exactly from an fp16 hi/lo split of x and
# gate_w (x.gw = xh.gh + xh.gl + xl.gh, error ~1e-7), so top-k selection
# matches the fp32 reference; the rest of the router is fp32 on DVE/Act.
# PSUM start_tensor_calc marks the whole 2KB zero region pending-zero, so
# only the very first matmul into the shared logits tile sets start=True.
#
# Routing weights are applied to the stage-1 activations with a deferred
# in-place scale pass on the Pool engine.
from contextlib import ExitStack

import numpy as np

import concourse.bass as bass
import concourse.tile as tile
from concourse import bacc, mybir
from concourse.masks import make_identity

F32 = mybir.dt.float32
F32R = mybir.dt.float32r
F16 = mybir.dt.float16
F8 = mybir.dt.float8e4
PM = mybir.MatmulPerfMode
AF = mybir.ActivationFunctionType
OP = mybir.AluOpType
AX = mybir.AxisListType

N_CORES = 8
T, H, I, E = 1024, 1024, 512, 32
E_LOC = E // N_CORES          # 4 experts per core
ISH = 2 * I // N_CORES        # 256 shared-intermediate rows per core
TT = T // 128                 # 8 token tiles
HK = H // 128                 # 8 h k-tiles
HP = HK // 2                  # 4 h k-tile PAIRS (fp8 DoubleRow, K=256)
IK = I // 128                 # 4 i-tiles per expert
SK = ISH // 128               # 2 shared i-tiles
TH = T // 512                 # 2 t halves (stage-1 rhs width)
NH = H // 512                 # 2 h halves (stage-3 rhs width)

SX = 4.0                      # fp8 scale on x
SW = 256.0                    # fp8 scale on gate/up weights
CINV = 1.0 / (SX * SW)        # descale folded into silu-scale / evacuation

# entry table: (kind, expert idx or None, #i-tiles); shared first so phase A
# can start before the router finishes (no routing weight needed).
ENTRIES = [("shared", None, SK)] + [("expert", e, IK) for e in range(E_LOC)]
N_ITILES = SK + E_LOC * IK    # 18 i-tiles total


def build_module(use_collective=True, num_devices=N_CORES):
    nc = bacc.Bacc("TRN2", target_bir_lowering=False, debug=False,
                   num_devices=num_devices)

    # router operands (fp16 exact-split path)
    xh_d = nc.dram_tensor("xh", [H, T], F16, kind="ExternalInput")
    xl_d = nc.dram_tensor("xl", [H, T], F16, kind="ExternalInput")
    ghl_d = nc.dram_tensor("ghl", [128, HK * 2 * E], F16, kind="ExternalInput")
    bias_d = nc.dram_tensor("bias", [128, E], F32, kind="ExternalInput")
    wselbc_d = nc.dram_tensor("wselbc", [E, E_LOC * 128], F32,
                              kind="ExternalInput")
    # stage-1 fp8 DoubleRow operands: x packed [hp][128, (j, T)] hi/lo,
    # gate/up packed [128, (s=hi/lo, hp, j, I)]
    x8h_d = nc.dram_tensor("x8h", [HP, 128, 2 * T], F8, kind="ExternalInput")
    x8l_d = nc.dram_tensor("x8l", [HP, 128, 2 * T], F8, kind="ExternalInput")
    wg_d = nc.dram_tensor("wg", [E_LOC, 128, 2 * HP * 2 * I], F8,
                          kind="ExternalInput")
    wu_d = nc.dram_tensor("wu", [E_LOC, 128, 2 * HP * 2 * I], F8,
                          kind="ExternalInput")
    sg_d = nc.dram_tensor("sg", [128, 2 * HP * 2 * ISH], F8,
                          kind="ExternalInput")
    su_d = nc.dram_tensor("su", [128, 2 * HP * 2 * ISH], F8,
                          kind="ExternalInput")
    # stage-3 fp16 down weights packed [128, (it, H)]
    wd_d = nc.dram_tensor("wd", [E_LOC, 128, IK * H], F16,
                          kind="ExternalInput")
    sd_d = nc.dram_tensor("sd", [128, SK * H], F16, kind="ExternalInput")
    out_rows = T // num_devices if use_collective else T
    out_d = nc.dram_tensor("out", [out_rows, H], F32, kind="ExternalOutput")

    with tile.TileContext(nc) as tc, ExitStack() as ctx:
        const = ctx.enter_context(tc.tile_pool(name="const", bufs=1))
        sbr = ctx.enter_context(tc.tile_pool(name="router", bufs=2))
        xpool = ctx.enter_context(tc.tile_pool(name="xt", bufs=1))
        x8pool = ctx.enter_context(tc.tile_pool(name="x8", bufs=1))
        xlp = ctx.enter_context(tc.tile_pool(name="xl", bufs=1))
        wgu_pool = ctx.enter_context(tc.tile_pool(name="wgu", bufs=1))
        wd_pool = ctx.enter_context(tc.tile_pool(name="wd", bufs=1))
        a_pool = ctx.enter_context(tc.tile_pool(name="ats", bufs=1))
        wb_pool = ctx.enter_context(tc.tile_pool(name="wb", bufs=1))
        tmp_pool = ctx.enter_context(tc.tile_pool(name="tmp", bufs=3))
        stg_pool = ctx.enter_context(tc.tile_pool(name="stg", bufs=2))
        dram = ctx.enter_context(tc.tile_pool(name="dram", bufs=1, space="DRAM"))

        ps_main = ctx.enter_context(tc.tile_pool(name="ps_main", bufs=5,
                                                 space="PSUM"))
        ps_r = ctx.enter_context(tc.tile_pool(name="ps_r", bufs=2,
                                              space="PSUM"))
        ps_lg = ctx.enter_context(tc.tile_pool(name="ps_lg", bufs=1,
                                               space="PSUM"))

        ident_f = const.tile([128, 128], F32, name="ident_f")

        # ---- DMA plan ------------------------------------------------------
        # One serial DMA stream (~344 GB/s): shared fp8 weights, x fp8 pairs
        # (pace the first chains), e0 weights interleaved with the router's
        # fp16 x tiles, xl stream, router smalls, e1..e3, down weights, outs.
        sg_sb = wgu_pool.tile([128, 2 * HP * 2 * ISH], F8, name="sg_sb")
        nc.sync.dma_start(sg_sb[:], sg_d[:])
        x8h = [x8pool.tile([128, 2 * T], F8, name=f"x8h{hp}")
               for hp in range(HP)]
        x8l = [x8pool.tile([128, 2 * T], F8, name=f"x8l{hp}")
               for hp in range(HP)]
        nc.sync.dma_start(x8h[0][:], x8h_d[0])
        nc.sync.dma_start(x8l[0][:], x8l_d[0])
        su_sb = wgu_pool.tile([128, 2 * HP * 2 * ISH], F8, name="su_sb")
        nc.sync.dma_start(su_sb[:], su_d[:])
        for hp in range(1, HP):
            nc.sync.dma_start(x8h[hp][:], x8h_d[hp])
            nc.sync.dma_start(x8l[hp][:], x8l_d[hp])

        wg_sb, wu_sb = [sg_sb], [su_sb]
        for e in range(E_LOC):
            g = wgu_pool.tile([128, 2 * HP * 2 * I], F8, name=f"wg{e}")
            u = wgu_pool.tile([128, 2 * HP * 2 * I], F8, name=f"wu{e}")
            wg_sb.append(g)
            wu_sb.append(u)
        nc.sync.dma_start(wg_sb[1][:], wg_d[0])
        nc.sync.dma_start(wu_sb[1][:], wu_d[0])
        ghl_sb = sbr.tile([128, HK * 2 * E], F16, name="ghl_sb")
        nc.sync.dma_start(ghl_sb[:], ghl_d[:])
        xt = [xpool.tile([128, T], F16, name=f"xt{ht}", tag="xt",
                         bufs=6) for ht in range(HK)]
        for ht in range(HK):
            nc.sync.dma_start(xt[ht][:], xh_d[ht * 128:(ht + 1) * 128, :])

        wd_sb = [wd_pool.tile([128, SK * H], F16, name="sd_sb")]
        for e in range(E_LOC):
            wd_sb.append(wd_pool.tile([128, IK * H], F16, name=f"wd{e}"))

        def late_dmas():
            # issued after the xl stream in queue order
            for e in range(1, E_LOC):
                nc.sync.dma_start(wg_sb[1 + e][:], wg_d[e])
                nc.sync.dma_start(wu_sb[1 + e][:], wu_d[e])
            nc.sync.dma_start(wd_sb[0][:], sd_d[:])
            for e in range(E_LOC):
                nc.sync.dma_start(wd_sb[1 + e][:], wd_d[e])

        make_identity(nc, ident_f[:])
        bias_bc = sbr.tile([128, E], F32, name="bias_bc")
        wselbc_sb = sbr.tile([E, E_LOC * 128], F32R, name="wselbc_sb")

        a_tiles = [a_pool.tile([128, T], F16, name=f"a{i}")
                   for i in range(N_ITILES)]
        a_base = {}
        off = 0
        for ei, (kind, e, ike) in enumerate(ENTRIES):
            a_base[ei] = off
            off += ike

        # ---- router: exact fp16-split logits -------------------------------
        lgall = ps_lg.tile([128, TT * E], F32, name="lgall")

        def gh_sl(ht):
            return ghl_sb[:, ht * 2 * E:ht * 2 * E + E]

        def gl_sl(ht):
            return ghl_sb[:, ht * 2 * E + E:(ht + 1) * 2 * E]

        def logits12_group(ht):
            # xh.gh + xh.gl terms (no xl dependency).  Only the very first
            # matmul into lgall's zero region may set start=True.
            for pi, rh in enumerate((gh_sl(ht), gl_sl(ht))):
                for tt in range(TT):
                    nc.tensor.matmul(
                        lgall[:, tt * E:(tt + 1) * E],
                        xt[ht][:, tt * 128:(tt + 1) * 128],
                        rh,
                        start=(ht == 0 and pi == 0 and tt == 0), stop=False,
                        skip_group_check=True)

        def logits3_group(ht):
            # xl.gh correction term
            xlt = xlp.tile([128, T], F16, name=f"xl{ht}", tag="xl", bufs=3)
            nc.sync.dma_start(xlt[:], xl_d[ht * 128:(ht + 1) * 128, :])
            for tt in range(TT):
                nc.tensor.matmul(
                    lgall[:, tt * E:(tt + 1) * E],
                    xlt[:, tt * 128:(tt + 1) * 128],
                    gh_sl(ht),
                    start=False, stop=(ht == HK - 1),
                    skip_group_check=True)

        # ---- stage 1: fp8 DoubleRow gate/up chains -------------------------
        # 12 matmuls per PSUM: (Wh.Xh, Wl.Xh, Wh.Xl) per h-pair hp=0..3.
        def w_sl(wt_, s, hp, it):
            # [p, (s, hp, j, i)] -> [p, 2, 128] slice for (s, hp, i-tile)
            v = wt_[:].rearrange("p (s hp j i) -> p s hp j i", s=2, hp=HP, j=2)
            return v[:, s, hp, :, it * 128:(it + 1) * 128]

        def x_sl(xt8, th):
            return xt8[:].rearrange("p (j t) -> p j t", j=2)[
                :, :, th * 512:(th + 1) * 512]

        def s1_chain(psum, wt_, it, th, ike):
            n = 0
            for hp in range(HP):
                # (Wh.Xh), (Wl.Xh), (Wh.Xl)
                for sw, xs in ((0, x8h[hp]), (1, x8h[hp]), (0, x8l[hp])):
                    nc.tensor.matmul(
                        psum[:], w_sl(wt_, sw, hp, it), x_sl(xs, th),
                        start=(n == 0), stop=(n == 3 * HP - 1),
                        perf_mode=PM.DoubleRow)
                    n += 1

        def stage2(ei, ab, it, th, gp, up):
            sg_t = tmp_pool.tile([128, 512], F32, name=f"sl{ei}_{th}_{it}",
                                 tag="silu")
            nc.scalar.activation(sg_t[:], gp[:], AF.Silu, scale=CINV)
            nc.vector.tensor_tensor(
                ab[it][:, th * 512:(th + 1) * 512], sg_t[:], up[:], OP.mult)

        # shared entry, th=0: hp-outer across all four PSUM chains so the PE
        # keeps pace with the arriving x8 pairs.
        def stage_a0(interleave_it):
            ike = SK
            ab = a_tiles[0:SK]
            gps = [ps_main.tile([128, 512], F32, name=f"gp0_0_{it}", tag="ps")
                   for it in range(ike)]
            ups = [ps_main.tile([128, 512], F32, name=f"up0_0_{it}", tag="ps")
                   for it in range(ike)]
            for hp in range(HP):
                terms = ((0, x8h[hp], 0), (1, x8h[hp], 1), (2, x8l[hp], 0))
                for it in range(ike):
                    for n3, xs, sw in terms:
                        nc.tensor.matmul(
                            gps[it][:], w_sl(sg_sb, sw, hp, it), x_sl(xs, 0),
                            start=(hp == 0 and n3 == 0),
                            stop=(hp == HP - 1 and n3 == 2),
                            perf_mode=PM.DoubleRow)
                    for n3, xs, sw in terms:
                        nc.tensor.matmul(
                            ups[it][:], w_sl(su_sb, sw, hp, it), x_sl(xs, 0),
                            start=(hp == 0 and n3 == 0),
                            stop=(hp == HP - 1 and n3 == 2),
                            perf_mode=PM.DoubleRow)
            for it in range(ike):
                stage2(0, ab, it, 0, gps[it], ups[it])
            for it in range(ike):
                gp = ps_main.tile([128, 512], F32, name=f"gp0_1_{it}",
                                  tag="ps")
                up = ps_main.tile([128, 512], F32, name=f"up0_1_{it}",
                                  tag="ps")
                s1_chain(gp, sg_sb, it, 1, ike)
                s1_chain(up, su_sb, it, 1, ike)
                stage2(0, ab, it, 1, gp, up)
                interleave_it(it)

        def stage_a(ei, interleave=None):
            kind, e, ike = ENTRIES[ei]
            wgt, wut = wg_sb[ei], wu_sb[ei]
            ab = a_tiles[a_base[ei]:a_base[ei] + ike]
            step = 0
            for th in range(TH):
                for it in range(ike):
                    gp = ps_main.tile([128, 512], F32,
                                      name=f"gp{ei}_{th}_{it}", tag="ps")
                    up = ps_main.tile([128, 512], F32,
                                      name=f"up{ei}_{th}_{it}", tag="ps")
                    s1_chain(gp, wgt, it, th, ike)
                    s1_chain(up, wut, it, th, ike)
                    stage2(ei, ab, it, th, gp, up)
                    if interleave is not None:
                        interleave(step)
                    step += 1

        # ---- router top-k math (DVE/Act only; transposes deferred) --------
        wt_tiles = []

        def routing_math(tt):
            lg = lgall[:, tt * E:(tt + 1) * E]
            S = sbr.tile([128, E], F32, name=f"S{tt}", tag="S")
            nc.scalar.activation(S[:], lg, AF.Sigmoid)
            SC = sbr.tile([128, E], F32, name=f"SC{tt}", tag="SC")
            nc.vector.tensor_tensor(SC[:], S[:], bias_bc[:], OP.add)
            topg = sbr.tile([128, E], F32, name=f"topg{tt}", tag="topg")
            for g in range(4):
                nc.vector.max(topg[:, 8 * g:8 * g + 8], SC[:, 8 * g:8 * g + 8])
            gs8 = sbr.tile([128, 8], F32, name=f"gs8{tt}", tag="gs8")
            nc.vector.memset(gs8[:], -1e30)
            tg = topg[:].rearrange("p (g k) -> p g k", k=8)
            nc.vector.tensor_tensor(gs8[:, 0:4], tg[:, :, 0], tg[:, :, 1],
                                    OP.add)
            gtop = sbr.tile([128, 8], F32, name=f"gtop{tt}", tag="gtop")
            nc.vector.max(gtop[:], gs8[:])
            gmask = sbr.tile([128, 4], F32, name=f"gmask{tt}", tag="gmask")
            nc.vector.tensor_scalar(gmask[:], gs8[:, 0:4], gtop[:, 1:2], None,
                                    OP.is_ge)
            SCm = sbr.tile([128, E], F32, name=f"SCm{tt}", tag="SCm")
            nc.vector.tensor_tensor(
                SCm[:].rearrange("p (g k) -> p g k", k=8),
                SC[:].rearrange("p (g k) -> p g k", k=8),
                gmask[:].rearrange("p (g k) -> p g k", k=1).broadcast_to(
                    [128, 4, 8]),
                OP.mult)
            etop = sbr.tile([128, 8], F32, name=f"etop{tt}", tag="etop")
            nc.vector.max(etop[:], SCm[:])
            sel = sbr.tile([128, E], F32, name=f"sel{tt}", tag="sel")
            nc.vector.tensor_scalar(sel[:], SCm[:], etop[:, 7:8], None,
                                    OP.is_ge)
            wr = sbr.tile([128, E], F32, name=f"wr{tt}", tag="wr")
            nc.vector.tensor_tensor(wr[:], S[:], sel[:], OP.mult)
            den = sbr.tile([128, 1], F32, name=f"den{tt}", tag="den")
            nc.vector.reduce_sum(den[:], wr[:], axis=AX.X)
            nc.vector.tensor_scalar(den[:], den[:], 1.0 / 2.5, None, OP.mult)
            dinv = sbr.tile([128, 1], F32, name=f"dinv{tt}", tag="dinv")
            nc.vector.reciprocal(dinv[:], den[:])
            wt = sbr.tile([128, E], F32, name=f"wt{tt}", tag="wt", bufs=8)
            nc.vector.tensor_scalar(wt[:], wr[:], dinv[:], None, OP.mult)
            wt_tiles.append(wt)

        # ================= emission schedule ===============================
        # Shared entry first (needs no routing weights); logits groups with
        # no xl dependency slot into its th=1 steps and expert-0's first
        # steps, the xl correction term into expert-0's later steps.
        stage_a0(interleave_it=lambda it: None)

        def e0_hook(s):
            if s < 4:
                logits12_group(2 * s)
                logits12_group(2 * s + 1)
            else:
                logits3_group(2 * (s - 4))
                logits3_group(2 * (s - 4) + 1)
            if s == 7:
                # routing math right behind the last logits write so the
                # static scheduler orders it ahead of e1's stage-2 work
                for tt in range(TT):
                    routing_math(tt)

        nc.sync.dma_start(bias_bc[:], bias_d[:])
        nc.sync.dma_start(wselbc_sb[:], wselbc_d[:].bitcast(F32R))
        stage_a(1, interleave=e0_hook)
        late_dmas()

        stage_a(2)

        # wt transposes + routing-weight broadcast rows; placed after e1's
        # stage-1 so the PE arrives here well after the DVE router finishes.
        wT_r = sbr.tile([E, T], F32R, name="wT_r")
        for tt in range(TT):
            p = ps_r.tile([128, 512], F32, name=f"wtp{tt}", tag="ps_r")
            nc.tensor.transpose(p[0:E, 0:128], wt_tiles[tt][:], ident_f[:])
            nc.vector.tensor_copy(wT_r[:, tt * 128:(tt + 1) * 128].bitcast(F32R),
                                  p[0:E, 0:128].bitcast(F32R))
        wb_tiles = []
        for e in range(E_LOC):
            wbt = wb_pool.tile([128, T], F16, name=f"wb{e}")
            for th in range(TH):
                p = ps_r.tile([128, 512], F32, name=f"wbp{e}_{th}", tag="ps_r")
                nc.tensor.matmul(p[:], wselbc_sb[:, e * 128:(e + 1) * 128],
                                 wT_r[:, th * 512:(th + 1) * 512],
                                 start=True, stop=True)
                nc.vector.tensor_copy(wbt[:, th * 512:(th + 1) * 512], p[:])
            wb_tiles.append(wbt)

        # deferred routing-weight scale: in-place on the Pool engine.
        def scale_pass(ei):
            kind, e, ike = ENTRIES[ei]
            ab = a_tiles[a_base[ei]:a_base[ei] + ike]
            for th in range(TH):
                for it in range(ike):
                    sl = ab[it][:, th * 512:(th + 1) * 512]
                    nc.gpsimd.tensor_tensor(
                        sl, sl, wb_tiles[e][:, th * 512:(th + 1) * 512],
                        OP.mult)

        scale_pass(1)
        scale_pass(2)
        for ei in range(3, len(ENTRIES)):
            stage_a(ei)
            scale_pass(ei)

        # ---- stage 3: one 18-matmul fp16 PSUM chain per output tile -------
        if use_collective:
            bin_t = dram.tile([T, H], F32, name="rsin")
            target = bin_t
        else:
            target = out_d
        for tt in range(TT):
            for hh in range(NH):
                op = ps_main.tile([128, 512], F32, name=f"o{tt}_{hh}",
                                  tag="ps")
                n = 0
                for ei, (kind, e, ike) in enumerate(ENTRIES):
                    for it in range(ike):
                        nc.tensor.matmul(
                            op[:],
                            a_tiles[a_base[ei] + it][:, tt * 128:(tt + 1) * 128],
                            wd_sb[ei][:, it * H + hh * 512:
                                      it * H + (hh + 1) * 512],
                            start=(n == 0), stop=(n == N_ITILES - 1))
                        n += 1
                st = stg_pool.tile([128, 512], F32, name=f"st{tt}_{hh}",
                                   tag="stg")
                nc.vector.tensor_scalar(st[:], op[:], CINV, None, OP.mult)
                nc.sync.dma_start(
                    target[tt * 128:(tt + 1) * 128, hh * 512:(hh + 1) * 512],
                    st[:])

        # ---- ReduceScatter + output ---------------------------------------
        if use_collective:
            bout_t = dram.tile([out_rows, H], F32, name="rsout")
            nc.gpsimd.collective_compute(
                "ReduceScatter", OP.add,
                replica_groups=[list(range(num_devices))],
                ins=[bin_t.opt()], outs=[bout_t.opt()])
            nc.sync.dma_start(out_d[:], bout_t[:])
    nc.compile()
    return nc


_NC_CACHE = {}


def _get_module():
    key = "spmd"
    if key not in _NC_CACHE:
        _NC_CACHE[key] = build_module(use_collective=True, num_devices=N_CORES)
    return _NC_CACHE[key]


def _pack_rows(a, blk=128):
    """[R, C] -> [128, (R//128) * C]: row-tile r128 layout for one-DMA loads."""
    r, c = a.shape
    return np.ascontiguousarray(
        a.reshape(r // blk, blk, c).transpose(1, 0, 2).reshape(blk, -1))


def _fp8_split(a):
    """fp8e4 hi/lo split: a ~= hi + lo (both float8_e4m3)."""
    import ml_dtypes
    hi = a.astype(ml_dtypes.float8_e4m3)
    lo = (a - hi.astype(np.float32)).astype(ml_dtypes.float8_e4m3)
    return hi, lo


def _pack_w8(wT):
    """[H, C] f32 (pre-transposed weight) -> [128, (s, hp, j, C)] fp8 pair."""
    h, c = wT.shape
    hi, lo = _fp8_split(wT * SW)
    arr = np.stack([np.asarray(hi), np.asarray(lo)])        # [2, H, C]
    arr = arr.reshape(2, HP, 2, 128, c).transpose(3, 0, 1, 2, 4)
    return np.ascontiguousarray(arr.reshape(128, 2 * HP * 2 * c))


def _pack_x8(xT):
    """[H, T] f32 -> hi/lo [HP, 128, (j, T)] fp8 DoubleRow layout."""
    hi, lo = _fp8_split(xT * SX)
    out = []
    for a in (hi, lo):
        b = np.asarray(a).reshape(HP, 2, 128, T).transpose(0, 2, 1, 3)
        out.append(np.ascontiguousarray(b.reshape(HP, 128, 2 * T)))
    return out


def make_in_maps(hidden_states, gate_w, gate_bias, expert_gate, expert_up,
                 expert_down, shared_gate, shared_up, shared_down):
    x = np.asarray(hidden_states, np.float32).reshape(T, H)
    xt = np.ascontiguousarray(x.T)                       # [H, T]
    xh = xt.astype(np.float16)
    xl = (xt - xh.astype(np.float32)).astype(np.float16)
    x8h, x8l = _pack_x8(xt)
    gwt = np.ascontiguousarray(np.asarray(gate_w, np.float32).T)  # [H, E]
    gh = gwt.astype(np.float16)
    gl = (gwt - gh.astype(np.float32)).astype(np.float16)
    # pack gh/gl as [128, (ht, {gh,gl}, E)]
    ghl = np.concatenate(
        [gh.reshape(HK, 128, E)[:, :, None, :],
         gl.reshape(HK, 128, E)[:, :, None, :]], axis=2)  # [HK,128,2,E]
    ghl = np.ascontiguousarray(
        ghl.transpose(1, 0, 2, 3).reshape(128, HK * 2 * E))
    bias = np.broadcast_to(
        np.asarray(gate_bias, np.float32).reshape(1, E), (128, E))
    bias = np.ascontiguousarray(bias)
    eg = np.asarray(expert_gate, np.float32)
    eu = np.asarray(expert_up, np.float32)
    ed = np.asarray(expert_down, np.float32)
    sgT = np.asarray(shared_gate, np.float32).T          # [H, 2I]
    suT = np.asarray(shared_up, np.float32).T            # [H, 2I]
    sd = np.asarray(shared_down, np.float32)             # [H, 2I]
    in_maps = []
    for c in range(N_CORES):
        lo, hi = c * E_LOC, (c + 1) * E_LOC
        wselbc = np.zeros((E, E_LOC * 128), np.float32)
        for j in range(E_LOC):
            wselbc[lo + j, j * 128:(j + 1) * 128] = 1.0
        wg = np.stack([_pack_w8(eg[lo + j].T) for j in range(E_LOC)])
        wu = np.stack([_pack_w8(eu[lo + j].T) for j in range(E_LOC)])
        wd = np.stack([_pack_rows(ed[lo + j].T.astype(np.float16))
                       for j in range(E_LOC)])           # [E_LOC,128,IK*H]
        in_maps.append({
            "xh": xh, "xl": xl, "x8h": x8h, "x8l": x8l,
            "ghl": ghl, "bias": bias, "wselbc": wselbc,
            "wg": wg, "wu": wu, "wd": wd,
            "sg": _pack_w8(np.ascontiguousarray(sgT[:, c * ISH:(c + 1) * ISH])),
            "su": _pack_w8(np.ascontiguousarray(suT[:, c * ISH:(c + 1) * ISH])),
            "sd": _pack_rows(np.ascontiguousarray(
                sd[:, c * ISH:(c + 1) * ISH].T).astype(np.float16)),
        })
    return in_maps


def kernel(hidden_states, gate_w, gate_bias, expert_gate, expert_up,
           expert_down, shared_gate, shared_up, shared_down):
    import os
    # The axon NTFF trace hook is absent in this container; make sure the
    # PJRT execute path never tries to use it.
    os.environ.setdefault("BASS_NEVER_TRACE", "1")
    from concourse.bass_utils import run_bass_kernel_spmd
    nc = _get_module()
    in_maps = make_in_maps(hidden_states, gate_w, gate_bias, expert_gate,
                           expert_up, expert_down, shared_gate, shared_up,
                           shared_down)
    res = run_bass_kernel_spmd(nc, in_maps, core_ids=list(range(N_CORES)))
    out = np.concatenate([np.asarray(res.results[c]["out"], np.float32)
                          for c in range(N_CORES)], axis=0)
    return out.reshape(np.asarray(hidden_states).shape)


# revision 30
# speedup vs baseline: 1.7533x; 1.0051x over previous
# DeepseekV3MoECalibrate Trainium2 kernel (8 NeuronCores, expert-parallel).
#
# Sharding: 32 experts -> 4 per core; shared expert split along the 2I=2048
# intermediate dim (256 rows per core); tokens replicated; partial outputs
# summed with an on-device ReduceScatter.
#
# All weights and the token matrix are pre-transposed AND pre-packed on the
# HOST into the exact [128, free] SBUF layouts the PE needs, so the
# TensorEngine runs nothing but full-rate matmuls (no on-device transposes,
# no weight PSUM-evacuation copies) and every weight matrix is a single
# large DMA (per-DMA queue overhead ~0.9us makes small transfers expensive).
#
# Stage-1 (gate/up projections) runs in fp8e4 DoubleRow perf mode (K=256
# per instruction, 0.5 cycles/row) using a hi/lo fp8 split of both operands:
#   W.X ~= Wh.Xh + Wl.Xh + Wh.Xl   (error ~0.2-0.4%, vs the 2e-2 gate)
# Operands are pre-scaled by powers of two on the host (x*4, w*256) to
# avoid the fp8 denormal range; the 1/1024 descale is applied exactly via
# the silu's input scale and folded out of the up-path at the stage-3
# PSUM evacuation (tensor_scalar instead of tensor_copy, same cost).
# Stage-3 (down projection) stays fp16: one 18-matmul PSUM accumulation
# chain per output tile (4 experts x 4 i-tiles + 2 shared i-tiles).
#
# Router logits are computed exactly from an fp16 hi/lo split of x and
# gate_w (x.gw = xh.gh + xh.gl + xl.gh, error ~1e-7), so top-k selection
# matches the fp32 reference; the rest of the router is fp32 on DVE/Act.
# PSUM start_tensor_calc marks the whole 2KB zero region pending-zero, so
# only the very first matmul into the shared logits tile sets start=True.
#
# Routing weights are applied to the stage-1 activations with a deferred
# in-place scale pass on the Pool engine.
from contextlib import ExitStack

import numpy as np

import concourse.bass as bass
import concourse.tile as tile
from concourse import bacc, mybir
from concourse.masks import make_identity

F32 = mybir.dt.float32
F32R = mybir.dt.float32r
F16 = mybir.dt.float16
F8 = mybir.dt.float8e4
PM = mybir.MatmulPerfMode
AF = mybir.ActivationFunctionType
OP = mybir.AluOpType
AX = mybir.AxisListType

N_CORES = 8
T, H, I, E = 1024, 1024, 512, 32
E_LOC = E // N_CORES          # 4 experts per core
ISH = 2 * I // N_CORES        # 256 shared-intermediate rows per core
TT = T // 128                 # 8 token tiles
HK = H // 128                 # 8 h k-tiles
HP = HK // 2                  # 4 h k-tile PAIRS (fp8 DoubleRow, K=256)
IK = I // 128                 # 4 i-tiles per expert
SK = ISH // 128               # 2 shared i-tiles
TH = T // 512                 # 2 t halves (stage-1 rhs width)
NH = H // 512                 # 2 h halves (stage-3 rhs width)

SX = 4.0                      # fp8 scale on x
SW = 256.0                    # fp8 scale on gate/up weights
CINV = 1.0 / (SX * SW)        # descale folded into silu-scale / evacuation

# entry table: (kind, expert idx or None, #i-tiles); shared first so phase A
# can start before the router finishes (no routing weight needed).
ENTRIES = [("shared", None, SK)] + [("expert", e, IK) for e in range(E_LOC)]
N_ITILES = SK + E_LOC * IK    # 18 i-tiles total


def build_module(use_collective=True, num_devices=N_CORES):
    nc = bacc.Bacc("TRN2", target_bir_lowering=False, debug=False,
                   num_devices=num_devices)

    # router operands (fp16 exact-split path)
    xh_d = nc.dram_tensor("xh", [H, T], F16, kind="ExternalInput")
    xl_d = nc.dram_tensor("xl", [H, T], F16, kind="ExternalInput")
    ghl_d = nc.dram_tensor("ghl", [128, HK * 2 * E], F16, kind="ExternalInput")
    bias_d = nc.dram_tensor("bias", [128, E], F32, kind="ExternalInput")
    wselbc_d = nc.dram_tensor("wselbc", [E, E_LOC * 128], F32,
                              kind="ExternalInput")
    # stage-1 fp8 DoubleRow operands: x packed [hp][128, (j, T)] hi/lo,
    # gate/up packed [128, (s=hi/lo, hp, j, I)]
    x8h_d = nc.dram_tensor("x8h", [HP, 128, 2 * T], F8, kind="ExternalInput")
    x8l_d = nc.dram_tensor("x8l", [HP, 128, 2 * T], F8, kind="ExternalInput")
    wg_d = nc.dram_tensor("wg", [E_LOC, 128, 2 * HP * 2 * I], F8,
                          kind="ExternalInput")
    wu_d = nc.dram_tensor("wu", [E_LOC, 128, 2 * HP * 2 * I], F8,
                          kind="ExternalInput")
    sg_d = nc.dram_tensor("sg", [128, 2 * HP * 2 * ISH], F8,
                          kind="ExternalInput")
    su_d = nc.dram_tensor("su", [128, 2 * HP * 2 * ISH], F8,
                          kind="ExternalInput")
    # stage-3 fp16 down weights packed [128, (it, H)]
    wd_d = nc.dram_tensor("wd", [E_LOC, 128, IK * H], F16,
                          kind="ExternalInput")
    sd_d = nc.dram_tensor("sd", [128, SK * H], F16, kind="ExternalInput")
    out_rows = T // num_devices if use_collective else T
    out_d = nc.dram_tensor("out", [out_rows, H], F32, kind="ExternalOutput")

    with tile.TileContext(nc) as tc, ExitStack() as ctx:
        const = ctx.enter_context(tc.tile_pool(name="const", bufs=1))
        sbr = ctx.enter_context(tc.tile_pool(name="router", bufs=2))
        xpool = ctx.enter_context(tc.tile_pool(name="xt", bufs=1))
        x8pool = ctx.enter_context(tc.tile_pool(name="x8", bufs=1))
        xlp = ctx.enter_context(tc.tile_pool(name="xl", bufs=1))
        wgu_pool = ctx.enter_context(tc.tile_pool(name="wgu", bufs=1))
        wd_pool = ctx.enter_context(tc.tile_pool(name="wd", bufs=1))
        a_pool = ctx.enter_context(tc.tile_pool(name="ats", bufs=1))
        wb_pool = ctx.enter_context(tc.tile_pool(name="wb", bufs=1))
        tmp_pool = ctx.enter_context(tc.tile_pool(name="tmp", bufs=3))
        stg_pool = ctx.enter_context(tc.tile_pool(name="stg", bufs=2))
        dram = ctx.enter_context(tc.tile_pool(name="dram", bufs=1, space="DRAM"))

        ps_main = ctx.enter_context(tc.tile_pool(name="ps_main", bufs=5,
                                                 space="PSUM"))
        ps_r = ctx.enter_context(tc.tile_pool(name="ps_r", bufs=2,
                                              space="PSUM"))
        ps_lg = ctx.enter_context(tc.tile_pool(name="ps_lg", bufs=1,
                                               space="PSUM"))

        ident_f = const.tile([128, 128], F32, name="ident_f")

        # ---- DMA plan ------------------------------------------------------
        # One serial DMA stream (~344 GB/s): shared fp8 weights, x fp8 pairs
        # (pace the first chains), e0 weights interleaved with the router's
        # fp16 x tiles, xl stream, router smalls, e1..e3, down weights, outs.
        sg_sb = wgu_pool.tile([128, 2 * HP * 2 * ISH], F8, name="sg_sb")
        nc.sync.dma_start(sg_sb[:], sg_d[:])
        x8h = [x8pool.tile([128, 2 * T], F8, name=f"x8h{hp}")
               for hp in range(HP)]
        x8l = [x8pool.tile([128, 2 * T], F8, name=f"x8l{hp}")
               for hp in range(HP)]
        nc.sync.dma_start(x8h[0][:], x8h_d[0])
        nc.sync.dma_start(x8l[0][:], x8l_d[0])
        su_sb = wgu_pool.tile([128, 2 * HP * 2 * ISH], F8, name="su_sb")
        nc.sync.dma_start(su_sb[:], su_d[:])
        for hp in range(1, HP):
            nc.sync.dma_start(x8h[hp][:], x8h_d[hp])
            nc.sync.dma_start(x8l[hp][:], x8l_d[hp])

        wg_sb, wu_sb = [sg_sb], [su_sb]
        for e in range(E_LOC):
            g = wgu_pool.tile([128, 2 * HP * 2 * I], F8, name=f"wg{e}")
            u = wgu_pool.tile([128, 2 * HP * 2 * I], F8, name=f"wu{e}")
            wg_sb.append(g)
            wu_sb.append(u)
        nc.sync.dma_start(wg_sb[1][:], wg_d[0])
        ghl_sb = sbr.tile([128, HK * 2 * E], F16, name="ghl_sb")
        nc.sync.dma_start(ghl_sb[:], ghl_d[:])
        xt = [xpool.tile([128, T], F16, name=f"xt{ht}", tag="xt",
                         bufs=6) for ht in range(HK)]
        for ht in range(4):
            nc.sync.dma_start(xt[ht][:], xh_d[ht * 128:(ht + 1) * 128, :])
        nc.sync.dma_start(wu_sb[1][:], wu_d[0])
        for ht in range(4, HK):
            nc.sync.dma_start(xt[ht][:], xh_d[ht * 128:(ht + 1) * 128, :])

        wd_sb = [wd_pool.tile([128, SK * H], F16, name="sd_sb")]
        for e in range(E_LOC):
            wd_sb.append(wd_pool.tile([128, IK * H], F16, name=f"wd{e}"))

        def late_dmas():
            # issued after the xl stream in queue order
            for e in range(1, E_LOC):
                nc.sync.dma_start(wg_sb[1 + e][:], wg_d[e])
                nc.sync.dma_start(wu_sb[1 + e][:], wu_d[e])
            nc.sync.dma_start(wd_sb[0][:], sd_d[:])
            for e in range(E_LOC):
                nc.sync.dma_start(wd_sb[1 + e][:], wd_d[e])

        make_identity(nc, ident_f[:])
        bias_bc = sbr.tile([128, E], F32, name="bias_bc")
        wselbc_sb = sbr.tile([E, E_LOC * 128], F32R, name="wselbc_sb")

        a_tiles = [a_pool.tile([128, T], F16, name=f"a{i}")
                   for i in range(N_ITILES)]
        a_base = {}
        off = 0
        for ei, (kind, e, ike) in enumerate(ENTRIES):
            a_base[ei] = off
            off += ike

        # ---- router: exact fp16-split logits -------------------------------
        lgall = ps_lg.tile([128, TT * E], F32, name="lgall")

        def gh_sl(ht):
            return ghl_sb[:, ht * 2 * E:ht * 2 * E + E]

        def gl_sl(ht):
            return ghl_sb[:, ht * 2 * E + E:(ht + 1) * 2 * E]

        def logits12_group(ht):
            # xh.gh + xh.gl terms (no xl dependency).  Only the very first
            # matmul into lgall's zero region may set start=True.
            for pi, rh in enumerate((gh_sl(ht), gl_sl(ht))):
                for tt in range(TT):
                    nc.tensor.matmul(
                        lgall[:, tt * E:(tt + 1) * E],
                        xt[ht][:, tt * 128:(tt + 1) * 128],
                        rh,
                        start=(ht == 0 and pi == 0 and tt == 0), stop=False,
                        skip_group_check=True)

        def logits3_group(ht):
            # xl.gh correction term
            xlt = xlp.tile([128, T], F16, name=f"xl{ht}", tag="xl", bufs=3)
            nc.sync.dma_start(xlt[:], xl_d[ht * 128:(ht + 1) * 128, :])
            for tt in range(TT):
                nc.tensor.matmul(
                    lgall[:, tt * E:(tt + 1) * E],
                    xlt[:, tt * 128:(tt + 1) * 128],
                    gh_sl(ht),
                    start=False, stop=(ht == HK - 1),
                    skip_group_check=True)

        # ---- stage 1: fp8 DoubleRow gate/up chains -------------------------
        # 12 matmuls per PSUM: (Wh.Xh, Wl.Xh, Wh.Xl) per h-pair hp=0..3.
        def w_sl(wt_, s, hp, it):
            # [p, (s, hp, j, i)] -> [p, 2, 128] slice for (s, hp, i-tile)
            v = wt_[:].rearrange("p (s hp j i) -> p s hp j i", s=2, hp=HP, j=2)
            return v[:, s, hp, :, it * 128:(it + 1) * 128]

        def x_sl(xt8, th):
            return xt8[:].rearrange("p (j t) -> p j t", j=2)[
                :, :, th * 512:(th + 1) * 512]

        def s1_chain(psum, wt_, it, th, ike):
            n = 0
            for hp in range(HP):
                # (Wh.Xh), (Wl.Xh), (Wh.Xl)
                for sw, xs in ((0, x8h[hp]), (1, x8h[hp]), (0, x8l[hp])):
                    nc.tensor.matmul(
                        psum[:], w_sl(wt_, sw, hp, it), x_sl(xs, th),
                        start=(n == 0), stop=(n == 3 * HP - 1),
                        perf_mode=PM.DoubleRow)
                    n += 1

        def stage2(ei, ab, it, th, gp, up):
            sg_t = tmp_pool.tile([128, 512], F32, name=f"sl{ei}_{th}_{it}",
                                 tag="silu")
            nc.scalar.activation(sg_t[:], gp[:], AF.Silu, scale=CINV)
            nc.vector.tensor_tensor(
                ab[it][:, th * 512:(th + 1) * 512], sg_t[:], up[:], OP.mult)

        # shared entry, th=0: hp-outer across all four PSUM chains so the PE
        # keeps pace with the arriving x8 pairs.
        def stage_a0(interleave_it):
            ike = SK
            ab = a_tiles[0:SK]
            gps = [ps_main.tile([128, 512], F32, name=f"gp0_0_{it}", tag="ps")
                   for it in range(ike)]
            ups = [ps_main.tile([128, 512], F32, name=f"up0_0_{it}", tag="ps")
                   for it in range(ike)]
            for hp in range(HP):
                terms = ((0, x8h[hp], 0), (1, x8h[hp], 1), (2, x8l[hp], 0))
                for it in range(ike):
                    for n3, xs, sw in terms:
                        nc.tensor.matmul(
                            gps[it][:], w_sl(sg_sb, sw, hp, it), x_sl(xs, 0),
                            start=(hp == 0 and n3 == 0),
                            stop=(hp == HP - 1 and n3 == 2),
                            perf_mode=PM.DoubleRow)
                    for n3, xs, sw in terms:
                        nc.tensor.matmul(
                            ups[it][:], w_sl(su_sb, sw, hp, it), x_sl(xs, 0),
                            start=(hp == 0 and n3 == 0),
                            stop=(hp == HP - 1 and n3 == 2),
                            perf_mode=PM.DoubleRow)
            for it in range(ike):
                stage2(0, ab, it, 0, gps[it], ups[it])
            for it in range(ike):
                gp = ps_main.tile([128, 512], F32, name=f"gp0_1_{it}",
                                  tag="ps")
                up = ps_main.tile([128, 512], F32, name=f"up0_1_{it}",
                                  tag="ps")
                s1_chain(gp, sg_sb, it, 1, ike)
                s1_chain(up, su_sb, it, 1, ike)
                stage2(0, ab, it, 1, gp, up)
                interleave_it(it)

        def stage_a(ei, interleave=None, gu_split_th0=False):
            kind, e, ike = ENTRIES[ei]
            wgt, wut = wg_sb[ei], wu_sb[ei]
            ab = a_tiles[a_base[ei]:a_base[ei] + ike]
            step = 0
            for th in range(TH):
                if gu_split_th0 and th == 0:
                    gps = []
                    for it in range(ike):
                        gp = ps_main.tile([128, 512], F32,
                                          name=f"gp{ei}_0_{it}", tag="ps")
                        s1_chain(gp, wgt, it, 0, ike)
                        gps.append(gp)
                        if interleave is not None:
                            interleave(step)
                        step += 1
                    for it in range(ike):
                        up = ps_main.tile([128, 512], F32,
                                          name=f"up{ei}_0_{it}", tag="ps")
                        s1_chain(up, wut, it, 0, ike)
                        stage2(ei, ab, it, 0, gps[it], up)
                        if interleave is not None:
                            interleave(step)
                        step += 1
                    continue
                for it in range(ike):
                    gp = ps_main.tile([128, 512], F32,
                                      name=f"gp{ei}_{th}_{it}", tag="ps")
                    up = ps_main.tile([128, 512], F32,
                                      name=f"up{ei}_{th}_{it}", tag="ps")
                    s1_chain(gp, wgt, it, th, ike)
                    s1_chain(up, wut, it, th, ike)
                    stage2(ei, ab, it, th, gp, up)
                    if interleave is not None:
                        interleave(step)
                    step += 1

        # ---- router top-k math (DVE/Act only; transposes deferred) --------
        wt_tiles = []

        def routing_math(tt):
            lg = lgall[:, tt * E:(tt + 1) * E]
            S = sbr.tile([128, E], F32, name=f"S{tt}", tag="S")
            nc.scalar.activation(S[:], lg, AF.Sigmoid)
            SC = sbr.tile([128, E], F32, name=f"SC{tt}", tag="SC")
            nc.vector.tensor_tensor(SC[:], S[:], bias_bc[:], OP.add)
            topg = sbr.tile([128, E], F32, name=f"topg{tt}", tag="topg")
            for g in range(4):
                nc.vector.max(topg[:, 8 * g:8 * g + 8], SC[:, 8 * g:8 * g + 8])
            gs8 = sbr.tile([128, 8], F32, name=f"gs8{tt}", tag="gs8")
            nc.vector.memset(gs8[:], -1e30)
            tg = topg[:].rearrange("p (g k) -> p g k", k=8)
            nc.vector.tensor_tensor(gs8[:, 0:4], tg[:, :, 0], tg[:, :, 1],
                                    OP.add)
            gtop = sbr.tile([128, 8], F32, name=f"gtop{tt}", tag="gtop")
            nc.vector.max(gtop[:], gs8[:])
            gmask = sbr.tile([128, 4], F32, name=f"gmask{tt}", tag="gmask")
            nc.vector.tensor_scalar(gmask[:], gs8[:, 0:4], gtop[:, 1:2], None,
                                    OP.is_ge)
            SCm = sbr.tile([128, E], F32, name=f"SCm{tt}", tag="SCm")
            nc.vector.tensor_tensor(
                SCm[:].rearrange("p (g k) -> p g k", k=8),
                SC[:].rearrange("p (g k) -> p g k", k=8),
                gmask[:].rearrange("p (g k) -> p g k", k=1).broadcast_to(
                    [128, 4, 8]),
                OP.mult)
            etop = sbr.tile([128, 8], F32, name=f"etop{tt}", tag="etop")
            nc.vector.max(etop[:], SCm[:])
            sel = sbr.tile([128, E], F32, name=f"sel{tt}", tag="sel")
            nc.vector.tensor_scalar(sel[:], SCm[:], etop[:, 7:8], None,
                                    OP.is_ge)
            wr = sbr.tile([128, E], F32, name=f"wr{tt}", tag="wr")
            nc.vector.tensor_tensor(wr[:], S[:], sel[:], OP.mult)
            den = sbr.tile([128, 1], F32, name=f"den{tt}", tag="den")
            nc.vector.reduce_sum(den[:], wr[:], axis=AX.X)
            # the x2.5 routed scaling is folded into wselbc on the host
            dinv = sbr.tile([128, 1], F32, name=f"dinv{tt}", tag="dinv")
            nc.vector.reciprocal(dinv[:], den[:])
            wt = sbr.tile([128, E], F32, name=f"wt{tt}", tag="wt", bufs=8)
            nc.vector.tensor_scalar(wt[:], wr[:], dinv[:], None, OP.mult)
            wt_tiles.append(wt)

        # ================= emission schedule ===============================
        # Shared entry first (needs no routing weights); logits groups with
        # no xl dependency slot into its th=1 steps and expert-0's first
        # steps, the xl correction term into expert-0's later steps.
        stage_a0(interleave_it=lambda it: None)

        def e0_hook(s):
            if s < 8:
                logits12_group(s)
            elif s < 12:
                logits3_group(2 * (s - 8))
                logits3_group(2 * (s - 8) + 1)
            if s == 11:
                # routing math right behind the last logits write so the
                # static scheduler orders it ahead of e1's stage-2 work
                for tt in range(TT):
                    routing_math(tt)

        nc.sync.dma_start(bias_bc[:], bias_d[:])
        nc.sync.dma_start(wselbc_sb[:], wselbc_d[:].bitcast(F32R))
        stage_a(1, interleave=e0_hook, gu_split_th0=True)
        late_dmas()

        stage_a(2)

        # wt transposes + routing-weight broadcast rows; placed after e1's
        # stage-1 so the PE arrives here well after the DVE router finishes.
        wT_r = sbr.tile([E, T], F32R, name="wT_r")
        for tt in range(TT):
            p = ps_r.tile([128, 512], F32, name=f"wtp{tt}", tag="ps_r")
            nc.tensor.transpose(p[0:E, 0:128], wt_tiles[tt][:], ident_f[:])
            nc.vector.tensor_copy(wT_r[:, tt * 128:(tt + 1) * 128].bitcast(F32R),
                                  p[0:E, 0:128].bitcast(F32R))
        wb_tiles = []
        for e in range(E_LOC):
            wbt = wb_pool.tile([128, T], F16, name=f"wb{e}")
            for th in range(TH):
                p = ps_r.tile([128, 512], F32, name=f"wbp{e}_{th}", tag="ps_r")
                nc.tensor.matmul(p[:], wselbc_sb[:, e * 128:(e + 1) * 128],
                                 wT_r[:, th * 512:(th + 1) * 512],
                                 start=True, stop=True)
                nc.vector.tensor_copy(wbt[:, th * 512:(th + 1) * 512], p[:])
            wb_tiles.append(wbt)

        # deferred routing-weight scale: in-place on the Pool engine.
        def scale_pass(ei):
            kind, e, ike = ENTRIES[ei]
            ab = a_tiles[a_base[ei]:a_base[ei] + ike]
            for th in range(TH):
                for it in range(ike):
                    sl = ab[it][:, th * 512:(th + 1) * 512]
                    nc.gpsimd.tensor_tensor(
                        sl, sl, wb_tiles[e][:, th * 512:(th + 1) * 512],
                        OP.mult)

        scale_pass(1)
        scale_pass(2)
        for ei in range(3, len(ENTRIES)):
            stage_a(ei)
            scale_pass(ei)

        # ---- stage 3: one 18-matmul fp16 PSUM chain per output tile -------
        if use_collective:
            bin_t = dram.tile([T, H], F32, name="rsin")
            target = bin_t
        else:
            target = out_d
        for tt in range(TT):
            for hh in range(NH):
                op = ps_main.tile([128, 512], F32, name=f"o{tt}_{hh}",
                                  tag="ps")
                n = 0
                for ei, (kind, e, ike) in enumerate(ENTRIES):
                    for it in range(ike):
                        nc.tensor.matmul(
                            op[:],
                            a_tiles[a_base[ei] + it][:, tt * 128:(tt + 1) * 128],
                            wd_sb[ei][:, it * H + hh * 512:
                                      it * H + (hh + 1) * 512],
                            start=(n == 0), stop=(n == N_ITILES - 1))
                        n += 1
                st = stg_pool.tile([128, 512], F32, name=f"st{tt}_{hh}",
                                   tag="stg")
                nc.vector.tensor_scalar(st[:], op[:], CINV, None, OP.mult)
                nc.sync.dma_start(
                    target[tt * 128:(tt + 1) * 128, hh * 512:(hh + 1) * 512],
                    st[:])

        # ---- ReduceScatter + output ---------------------------------------
        if use_collective:
            bout_t = dram.tile([out_rows, H], F32, name="rsout")
            nc.gpsimd.collective_compute(
                "ReduceScatter", OP.add,
                replica_groups=[list(range(num_devices))],
                ins=[bin_t.opt()], outs=[bout_t.opt()])
            nc.sync.dma_start(out_d[:], bout_t[:])
    nc.compile()
    return nc


_NC_CACHE = {}


def _get_module():
    key = "spmd"
    if key not in _NC_CACHE:
        _NC_CACHE[key] = build_module(use_collective=True, num_devices=N_CORES)
    return _NC_CACHE[key]


def _pack_rows(a, blk=128):
    """[R, C] -> [128, (R//128) * C]: row-tile r128 layout for one-DMA loads."""
    r, c = a.shape
    return np.ascontiguousarray(
        a.reshape(r // blk, blk, c).transpose(1, 0, 2).reshape(blk, -1))


def _fp8_split(a):
    """fp8e4 hi/lo split: a ~= hi + lo (both float8_e4m3)."""
    import ml_dtypes
    hi = a.astype(ml_dtypes.float8_e4m3)
    lo = (a - hi.astype(np.float32)).astype(ml_dtypes.float8_e4m3)
    return hi, lo


def _pack_w8(wT):
    """[H, C] f32 (pre-transposed weight) -> [128, (s, hp, j, C)] fp8 pair."""
    h, c = wT.shape
    hi, lo = _fp8_split(wT * SW)
    arr = np.stack([np.asarray(hi), np.asarray(lo)])        # [2, H, C]
    arr = arr.reshape(2, HP, 2, 128, c).transpose(3, 0, 1, 2, 4)
    return np.ascontiguousarray(arr.reshape(128, 2 * HP * 2 * c))


def _pack_x8(xT):
    """[H, T] f32 -> hi/lo [HP, 128, (j, T)] fp8 DoubleRow layout."""
    hi, lo = _fp8_split(xT * SX)
    out = []
    for a in (hi, lo):
        b = np.asarray(a).reshape(HP, 2, 128, T).transpose(0, 2, 1, 3)
        out.append(np.ascontiguousarray(b.reshape(HP, 128, 2 * T)))
    return out


def make_in_maps(hidden_states, gate_w, gate_bias, expert_gate, expert_up,
                 expert_down, shared_gate, shared_up, shared_down):
    x = np.asarray(hidden_states, np.float32).reshape(T, H)
    xt = np.ascontiguousarray(x.T)                       # [H, T]
    xh = xt.astype(np.float16)
    xl = (xt - xh.astype(np.float32)).astype(np.float16)
    x8h, x8l = _pack_x8(xt)
    gwt = np.ascontiguousarray(np.asarray(gate_w, np.float32).T)  # [H, E]
    gh = gwt.astype(np.float16)
    gl = (gwt - gh.astype(np.float32)).astype(np.float16)
    # pack gh/gl as [128, (ht, {gh,gl}, E)]
    ghl = np.concatenate(
        [gh.reshape(HK, 128, E)[:, :, None, :],
         gl.reshape(HK, 128, E)[:, :, None, :]], axis=2)  # [HK,128,2,E]
    ghl = np.ascontiguousarray(
        ghl.transpose(1, 0, 2, 3).reshape(128, HK * 2 * E))
    bias = np.broadcast_to(
        np.asarray(gate_bias, np.float32).reshape(1, E), (128, E))
    bias = np.ascontiguousarray(bias)
    eg = np.asarray(expert_gate, np.float32)
    eu = np.asarray(expert_up, np.float32)
    ed = np.asarray(expert_down, np.float32)
    sgT = np.asarray(shared_gate, np.float32).T          # [H, 2I]
    suT = np.asarray(shared_up, np.float32).T            # [H, 2I]
    sd = np.asarray(shared_down, np.float32)             # [H, 2I]
    in_maps = []
    for c in range(N_CORES):
        lo, hi = c * E_LOC, (c + 1) * E_LOC
        wselbc = np.zeros((E, E_LOC * 128), np.float32)
        for j in range(E_LOC):
            wselbc[lo + j, j * 128:(j + 1) * 128] = 2.5
        wg = np.stack([_pack_w8(eg[lo + j].T) for j in range(E_LOC)])
        wu = np.stack([_pack_w8(eu[lo + j].T) for j in range(E_LOC)])
        wd = np.stack([_pack_rows(ed[lo + j].T.astype(np.float16))
                       for j in range(E_LOC)])           # [E_LOC,128,IK*H]
        in_maps.append({
            "xh": xh, "xl": xl, "x8h": x8h, "x8l": x8l,
            "ghl": ghl, "bias": bias, "wselbc": wselbc,
            "wg": wg, "wu": wu, "wd": wd,
            "sg": _pack_w8(np.ascontiguousarray(sgT[:, c * ISH:(c + 1) * ISH])),
            "su": _pack_w8(np.ascontiguousarray(suT[:, c * ISH:(c + 1) * ISH])),
            "sd": _pack_rows(np.ascontiguousarray(
                sd[:, c * ISH:(c + 1) * ISH].T).astype(np.float16)),
        })
    return in_maps


def kernel(hidden_states, gate_w, gate_bias, expert_gate, expert_up,
           expert_down, shared_gate, shared_up, shared_down):
    import os
    # The axon NTFF trace hook is absent in this container; make sure the
    # PJRT execute path never tries to use it.
    os.environ.setdefault("BASS_NEVER_TRACE", "1")
    from concourse.bass_utils import run_bass_kernel_spmd
    nc = _get_module()
    in_maps = make_in_maps(hidden_states, gate_w, gate_bias, expert_gate,
                           expert_up, expert_down, shared_gate, shared_up,
                           shared_down)
    res = run_bass_kernel_spmd(nc, in_maps, core_ids=list(range(N_CORES)))
    out = np.concatenate([np.asarray(res.results[c]["out"], np.float32)
                          for c in range(N_CORES)], axis=0)
    return out.reshape(np.asarray(hidden_states).shape)


# revision 34
# speedup vs baseline: 1.7573x; 1.0023x over previous
# DeepseekV3MoECalibrate Trainium2 kernel (8 NeuronCores, expert-parallel).
#
# Sharding: 32 experts -> 4 per core; shared expert split along the 2I=2048
# intermediate dim (256 rows per core); tokens replicated; partial outputs
# summed with an on-device ReduceScatter.
#
# All weights and the token matrix are pre-transposed AND pre-packed on the
# HOST into the exact [128, free] SBUF layouts the PE needs, so the
# TensorEngine runs nothing but full-rate matmuls (no on-device transposes,
# no weight PSUM-evacuation copies) and every weight matrix is a single
# large DMA (per-DMA queue overhead ~0.9us makes small transfers expensive).
#
# Stage-1 (gate/up projections) runs in fp8e4 DoubleRow perf mode (K=256
# per instruction, 0.5 cycles/row) using a hi/lo fp8 split of both operands:
#   W.X ~= Wh.Xh + Wl.Xh + Wh.Xl   (error ~0.2-0.4%, vs the 2e-2 gate)
# Operands are pre-scaled by powers of two on the host (x*4, w*256) to
# avoid the fp8 denormal range; the 1/1024 descale is applied exactly via
# the silu's input scale and folded out of the up-path at the stage-3
# PSUM evacuation (tensor_scalar instead of tensor_copy, same cost).
# Stage-3 (down projection) stays fp16: one 18-matmul PSUM accumulation
# chain per output tile (4 experts x 4 i-tiles + 2 shared i-tiles).
#
# Router logits are computed exactly from an fp16 hi/lo split of x and
# gate_w (x.gw = xh.gh + xh.gl + xl.gh, error ~1e-7), so top-k selection
# matches the fp32 reference; the rest of the router is fp32 on DVE/Act.
# PSUM start_tensor_calc marks the whole 2KB zero region pending-zero, so
# only the very first matmul into the shared logits tile sets start=True.
#
# Routing weights are applied to the stage-1 activations with a deferred
# in-place scale pass on the Pool engine.
from contextlib import ExitStack

import numpy as np

import concourse.bass as bass
import concourse.tile as tile
from concourse import bacc, mybir
from concourse.masks import make_identity

F32 = mybir.dt.float32
F32R = mybir.dt.float32r
F16 = mybir.dt.float16
F8 = mybir.dt.float8e4
PM = mybir.MatmulPerfMode
AF = mybir.ActivationFunctionType
OP = mybir.AluOpType
AX = mybir.AxisListType

N_CORES = 8
T, H, I, E = 1024, 1024, 512, 32
E_LOC = E // N_CORES          # 4 experts per core
ISH = 2 * I // N_CORES        # 256 shared-intermediate rows per core
TT = T // 128                 # 8 token tiles
HK = H // 128                 # 8 h k-tiles
HP = HK // 2                  # 4 h k-tile PAIRS (fp8 DoubleRow, K=256)
IK = I // 128                 # 4 i-tiles per expert
SK = ISH // 128               # 2 shared i-tiles
TH = T // 512                 # 2 t halves (stage-1 rhs width)
NH = H // 512                 # 2 h halves (stage-3 rhs width)

SX = 4.0                      # fp8 scale on x
SW = 256.0                    # fp8 scale on gate/up weights
CINV = 1.0 / (SX * SW)        # descale folded into silu-scale / evacuation

# entry table: (kind, expert idx or None, #i-tiles); shared first so phase A
# can start before the router finishes (no routing weight needed).
ENTRIES = [("shared", None, SK)] + [("expert", e, IK) for e in range(E_LOC)]
N_ITILES = SK + E_LOC * IK    # 18 i-tiles total


def build_module(use_collective=True, num_devices=N_CORES):
    nc = bacc.Bacc("TRN2", target_bir_lowering=False, debug=False,
                   num_devices=num_devices)

    # router operands (fp16 exact-split path)
    xh_d = nc.dram_tensor("xh", [H, T], F16, kind="ExternalInput")
    xl_d = nc.dram_tensor("xl", [H, T], F16, kind="ExternalInput")
    ghl_d = nc.dram_tensor("ghl", [128, HK * 2 * E], F16, kind="ExternalInput")
    bias_d = nc.dram_tensor("bias", [128, E], F32, kind="ExternalInput")
    wselbc_d = nc.dram_tensor("wselbc", [E, E_LOC * 128], F32,
                              kind="ExternalInput")
    # stage-1 fp8 DoubleRow operands: x packed [hp][128, (j, T)] hi/lo,
    # gate/up packed [128, (s=hi/lo, hp, j, I)]
    x8h_d = nc.dram_tensor("x8h", [HP, 128, 2 * T], F8, kind="ExternalInput")
    x8l_d = nc.dram_tensor("x8l", [HP, 128, 2 * T], F8, kind="ExternalInput")
    wg_d = nc.dram_tensor("wg", [E_LOC, 128, 2 * HP * 2 * I], F8,
                          kind="ExternalInput")
    wu_d = nc.dram_tensor("wu", [E_LOC, 128, 2 * HP * 2 * I], F8,
                          kind="ExternalInput")
    sg_d = nc.dram_tensor("sg", [128, 2 * HP * 2 * ISH], F8,
                          kind="ExternalInput")
    su_d = nc.dram_tensor("su", [128, 2 * HP * 2 * ISH], F8,
                          kind="ExternalInput")
    # stage-3 fp16 down weights packed [128, (it, H)]
    wd_d = nc.dram_tensor("wd", [E_LOC, 128, IK * H], F16,
                          kind="ExternalInput")
    sd_d = nc.dram_tensor("sd", [128, SK * H], F16, kind="ExternalInput")
    out_rows = T // num_devices if use_collective else T
    out_d = nc.dram_tensor("out", [out_rows, H], F32, kind="ExternalOutput")

    with tile.TileContext(nc) as tc, ExitStack() as ctx:
        const = ctx.enter_context(tc.tile_pool(name="const", bufs=1))
        sbr = ctx.enter_context(tc.tile_pool(name="router", bufs=2))
        xpool = ctx.enter_context(tc.tile_pool(name="xt", bufs=1))
        x8pool = ctx.enter_context(tc.tile_pool(name="x8", bufs=1))
        xlp = ctx.enter_context(tc.tile_pool(name="xl", bufs=1))
        wgu_pool = ctx.enter_context(tc.tile_pool(name="wgu", bufs=1))
        wd_pool = ctx.enter_context(tc.tile_pool(name="wd", bufs=1))
        a_pool = ctx.enter_context(tc.tile_pool(name="ats", bufs=1))
        wb_pool = ctx.enter_context(tc.tile_pool(name="wb", bufs=1))
        tmp_pool = ctx.enter_context(tc.tile_pool(name="tmp", bufs=3))
        stg_pool = ctx.enter_context(tc.tile_pool(name="stg", bufs=2))
        dram = ctx.enter_context(tc.tile_pool(name="dram", bufs=1, space="DRAM"))

        ps_main = ctx.enter_context(tc.tile_pool(name="ps_main", bufs=5,
                                                 space="PSUM"))
        ps_r = ctx.enter_context(tc.tile_pool(name="ps_r", bufs=2,
                                              space="PSUM"))
        ps_lg = ctx.enter_context(tc.tile_pool(name="ps_lg", bufs=1,
                                               space="PSUM"))

        ident_f = const.tile([128, 128], F32, name="ident_f")

        # ---- DMA plan ------------------------------------------------------
        # One serial DMA stream (~344 GB/s): shared fp8 weights, x fp8 pairs
        # (pace the first chains), e0 weights interleaved with the router's
        # fp16 x tiles, xl stream, router smalls, e1..e3, down weights, outs.
        sg_sb = wgu_pool.tile([128, 2 * HP * 2 * ISH], F8, name="sg_sb")
        nc.sync.dma_start(sg_sb[:], sg_d[:])
        x8h = [x8pool.tile([128, 2 * T], F8, name=f"x8h{hp}")
               for hp in range(HP)]
        x8l = [x8pool.tile([128, 2 * T], F8, name=f"x8l{hp}")
               for hp in range(HP)]
        nc.sync.dma_start(x8h[0][:], x8h_d[0])
        nc.sync.dma_start(x8l[0][:], x8l_d[0])
        su_sb = wgu_pool.tile([128, 2 * HP * 2 * ISH], F8, name="su_sb")
        nc.sync.dma_start(su_sb[:], su_d[:])
        for hp in range(1, HP):
            nc.sync.dma_start(x8h[hp][:], x8h_d[hp])
            nc.sync.dma_start(x8l[hp][:], x8l_d[hp])

        wg_sb, wu_sb = [sg_sb], [su_sb]
        for e in range(E_LOC):
            g = wgu_pool.tile([128, 2 * HP * 2 * I], F8, name=f"wg{e}")
            u = wgu_pool.tile([128, 2 * HP * 2 * I], F8, name=f"wu{e}")
            wg_sb.append(g)
            wu_sb.append(u)
        nc.sync.dma_start(wg_sb[1][:], wg_d[0])
        ghl_sb = sbr.tile([128, HK * 2 * E], F16, name="ghl_sb")
        nc.sync.dma_start(ghl_sb[:], ghl_d[:])
        xt = [xpool.tile([128, T], F16, name=f"xt{ht}", tag="xt",
                         bufs=6) for ht in range(HK)]
        for ht in range(4):
            nc.sync.dma_start(xt[ht][:], xh_d[ht * 128:(ht + 1) * 128, :])
        nc.sync.dma_start(wu_sb[1][:], wu_d[0])
        for ht in range(4, HK):
            nc.sync.dma_start(xt[ht][:], xh_d[ht * 128:(ht + 1) * 128, :])

        wd_sb = [wd_pool.tile([128, SK * H], F16, name="sd_sb")]
        for e in range(E_LOC):
            wd_sb.append(wd_pool.tile([128, IK * H], F16, name=f"wd{e}"))

        def late_dmas():
            # issued after the xl stream in queue order
            for e in range(1, E_LOC):
                nc.sync.dma_start(wg_sb[1 + e][:], wg_d[e])
                nc.sync.dma_start(wu_sb[1 + e][:], wu_d[e])
            nc.sync.dma_start(wd_sb[0][:], sd_d[:])
            for e in range(E_LOC):
                nc.sync.dma_start(wd_sb[1 + e][:], wd_d[e])

        make_identity(nc, ident_f[:])
        bias_bc = sbr.tile([128, E], F32, name="bias_bc")
        wselbc_sb = sbr.tile([E, E_LOC * 128], F32R, name="wselbc_sb")

        a_tiles = [a_pool.tile([128, T], F16, name=f"a{i}")
                   for i in range(N_ITILES)]
        a_base = {}
        off = 0
        for ei, (kind, e, ike) in enumerate(ENTRIES):
            a_base[ei] = off
            off += ike

        # ---- router: exact fp16-split logits -------------------------------
        lgall = ps_lg.tile([128, TT * E], F32, name="lgall")

        def gh_sl(ht):
            return ghl_sb[:, ht * 2 * E:ht * 2 * E + E]

        def gl_sl(ht):
            return ghl_sb[:, ht * 2 * E + E:(ht + 1) * 2 * E]

        def logits12_group(ht):
            # xh.gh + xh.gl terms (no xl dependency).  Only the very first
            # matmul into lgall's zero region may set start=True.
            for pi, rh in enumerate((gh_sl(ht), gl_sl(ht))):
                for tt in range(TT):
                    nc.tensor.matmul(
                        lgall[:, tt * E:(tt + 1) * E],
                        xt[ht][:, tt * 128:(tt + 1) * 128],
                        rh,
                        start=(ht == 0 and pi == 0 and tt == 0), stop=False,
                        skip_group_check=True)

        def logits3_group(ht):
            # xl.gh correction term
            xlt = xlp.tile([128, T], F16, name=f"xl{ht}", tag="xl", bufs=3)
            nc.sync.dma_start(xlt[:], xl_d[ht * 128:(ht + 1) * 128, :])
            for tt in range(TT):
                nc.tensor.matmul(
                    lgall[:, tt * E:(tt + 1) * E],
                    xlt[:, tt * 128:(tt + 1) * 128],
                    gh_sl(ht),
                    start=False, stop=(ht == HK - 1),
                    skip_group_check=True)

        # ---- stage 1: fp8 DoubleRow gate/up chains -------------------------
        # 12 matmuls per PSUM: (Wh.Xh, Wl.Xh, Wh.Xl) per h-pair hp=0..3.
        def w_sl(wt_, s, hp, it):
            # [p, (s, hp, j, i)] -> [p, 2, 128] slice for (s, hp, i-tile)
            v = wt_[:].rearrange("p (s hp j i) -> p s hp j i", s=2, hp=HP, j=2)
            return v[:, s, hp, :, it * 128:(it + 1) * 128]

        def x_sl(xt8, th):
            return xt8[:].rearrange("p (j t) -> p j t", j=2)[
                :, :, th * 512:(th + 1) * 512]

        def s1_chain(psum, wt_, it, th, ike):
            n = 0
            for hp in range(HP):
                # (Wh.Xh), (Wl.Xh), (Wh.Xl)
                for sw, xs in ((0, x8h[hp]), (1, x8h[hp]), (0, x8l[hp])):
                    nc.tensor.matmul(
                        psum[:], w_sl(wt_, sw, hp, it), x_sl(xs, th),
                        start=(n == 0), stop=(n == 3 * HP - 1),
                        perf_mode=PM.DoubleRow)
                    n += 1

        def stage2(ei, ab, it, th, gp, up):
            sg_t = tmp_pool.tile([128, 512], F32, name=f"sl{ei}_{th}_{it}",
                                 tag="silu")
            nc.scalar.activation(sg_t[:], gp[:], AF.Silu, scale=CINV)
            nc.vector.tensor_tensor(
                ab[it][:, th * 512:(th + 1) * 512], sg_t[:], up[:], OP.mult)

        # shared entry, th=0: hp-outer across all four PSUM chains so the PE
        # keeps pace with the arriving x8 pairs.
        def stage_a0(interleave_it):
            ike = SK
            ab = a_tiles[0:SK]
            gps = [ps_main.tile([128, 512], F32, name=f"gp0_0_{it}", tag="ps")
                   for it in range(ike)]
            ups = [ps_main.tile([128, 512], F32, name=f"up0_0_{it}", tag="ps")
                   for it in range(ike)]
            for hp in range(HP):
                terms = ((0, x8h[hp], 0), (1, x8h[hp], 1), (2, x8l[hp], 0))
                for it in range(ike):
                    for n3, xs, sw in terms:
                        nc.tensor.matmul(
                            gps[it][:], w_sl(sg_sb, sw, hp, it), x_sl(xs, 0),
                            start=(hp == 0 and n3 == 0),
                            stop=(hp == HP - 1 and n3 == 2),
                            perf_mode=PM.DoubleRow)
                    for n3, xs, sw in terms:
                        nc.tensor.matmul(
                            ups[it][:], w_sl(su_sb, sw, hp, it), x_sl(xs, 0),
                            start=(hp == 0 and n3 == 0),
                            stop=(hp == HP - 1 and n3 == 2),
                            perf_mode=PM.DoubleRow)
            for it in range(ike):
                stage2(0, ab, it, 0, gps[it], ups[it])
            for it in range(ike):
                gp = ps_main.tile([128, 512], F32, name=f"gp0_1_{it}",
                                  tag="ps")
                up = ps_main.tile([128, 512], F32, name=f"up0_1_{it}",
                                  tag="ps")
                s1_chain(gp, sg_sb, it, 1, ike)
                s1_chain(up, su_sb, it, 1, ike)
                stage2(0, ab, it, 1, gp, up)
                interleave_it(it)

        def stage_a(ei, interleave=None, gu_split_th0=False):
            kind, e, ike = ENTRIES[ei]
            wgt, wut = wg_sb[ei], wu_sb[ei]
            ab = a_tiles[a_base[ei]:a_base[ei] + ike]
            step = 0
            for th in range(TH):
                if gu_split_th0 and th == 0:
                    gps = []
                    for it in range(ike):
                        gp = ps_main.tile([128, 512], F32,
                                          name=f"gp{ei}_0_{it}", tag="ps")
                        s1_chain(gp, wgt, it, 0, ike)
                        gps.append(gp)
                        if interleave is not None:
                            interleave(step)
                        step += 1
                    for it in range(ike):
                        up = ps_main.tile([128, 512], F32,
                                          name=f"up{ei}_0_{it}", tag="ps")
                        s1_chain(up, wut, it, 0, ike)
                        stage2(ei, ab, it, 0, gps[it], up)
                        if interleave is not None:
                            interleave(step)
                        step += 1
                    continue
                for it in range(ike):
                    gp = ps_main.tile([128, 512], F32,
                                      name=f"gp{ei}_{th}_{it}", tag="ps")
                    up = ps_main.tile([128, 512], F32,
                                      name=f"up{ei}_{th}_{it}", tag="ps")
                    s1_chain(gp, wgt, it, th, ike)
                    s1_chain(up, wut, it, th, ike)
                    stage2(ei, ab, it, th, gp, up)
                    if interleave is not None:
                        interleave(step)
                    step += 1

        # ---- router top-k math (DVE/Act only; transposes deferred) --------
        wt_tiles = []

        def routing_math(tt):
            lg = lgall[:, tt * E:(tt + 1) * E]
            S = sbr.tile([128, E], F32, name=f"S{tt}", tag="S")
            nc.scalar.activation(S[:], lg, AF.Sigmoid)
            SC = sbr.tile([128, E], F32, name=f"SC{tt}", tag="SC")
            nc.vector.tensor_tensor(SC[:], S[:], bias_bc[:], OP.add)
            topg = sbr.tile([128, E], F32, name=f"topg{tt}", tag="topg")
            for g in range(4):
                nc.vector.max(topg[:, 8 * g:8 * g + 8], SC[:, 8 * g:8 * g + 8])
            gs8 = sbr.tile([128, 8], F32, name=f"gs8{tt}", tag="gs8")
            nc.vector.memset(gs8[:], -1e30)
            tg = topg[:].rearrange("p (g k) -> p g k", k=8)
            nc.vector.tensor_tensor(gs8[:, 0:4], tg[:, :, 0], tg[:, :, 1],
                                    OP.add)
            gtop = sbr.tile([128, 8], F32, name=f"gtop{tt}", tag="gtop")
            nc.vector.max(gtop[:], gs8[:])
            gmask = sbr.tile([128, 4], F32, name=f"gmask{tt}", tag="gmask")
            nc.vector.tensor_scalar(gmask[:], gs8[:, 0:4], gtop[:, 1:2], None,
                                    OP.is_ge)
            SCm = sbr.tile([128, E], F32, name=f"SCm{tt}", tag="SCm")
            nc.vector.tensor_tensor(
                SCm[:].rearrange("p (g k) -> p g k", k=8),
                SC[:].rearrange("p (g k) -> p g k", k=8),
                gmask[:].rearrange("p (g k) -> p g k", k=1).broadcast_to(
                    [128, 4, 8]),
                OP.mult)
            etop = sbr.tile([128, 8], F32, name=f"etop{tt}", tag="etop")
            nc.vector.max(etop[:], SCm[:])
            sel = sbr.tile([128, E], F32, name=f"sel{tt}", tag="sel")
            nc.vector.tensor_scalar(sel[:], SCm[:], etop[:, 7:8], None,
                                    OP.is_ge)
            wr = sbr.tile([128, E], F32, name=f"wr{tt}", tag="wr")
            nc.vector.tensor_tensor(wr[:], S[:], sel[:], OP.mult)
            den = sbr.tile([128, 1], F32, name=f"den{tt}", tag="den")
            nc.vector.reduce_sum(den[:], wr[:], axis=AX.X)
            # the x2.5 routed scaling is folded into wselbc on the host
            dinv = sbr.tile([128, 1], F32, name=f"dinv{tt}", tag="dinv")
            nc.vector.reciprocal(dinv[:], den[:])
            wt = sbr.tile([128, E], F32, name=f"wt{tt}", tag="wt", bufs=8)
            nc.vector.tensor_scalar(wt[:], wr[:], dinv[:], None, OP.mult)
            wt_tiles.append(wt)

        # ================= emission schedule ===============================
        # Shared entry first (needs no routing weights); logits groups with
        # no xl dependency slot into its th=1 steps and expert-0's first
        # steps, the xl correction term into expert-0's later steps.
        stage_a0(interleave_it=lambda it: None)

        def e0_hook(s):
            if s < 8:
                logits12_group(s)
            elif s < 12:
                logits3_group(2 * (s - 8))
                logits3_group(2 * (s - 8) + 1)
            if s == 11:
                # routing math right behind the last logits write so the
                # static scheduler orders it ahead of e1's stage-2 work
                for tt in range(TT):
                    routing_math(tt)

        nc.sync.dma_start(bias_bc[:], bias_d[:])
        nc.sync.dma_start(wselbc_sb[:], wselbc_d[:].bitcast(F32R))
        stage_a(1, interleave=e0_hook, gu_split_th0=True)
        late_dmas()

        stage_a(2)

        # wt transposes + routing-weight broadcast rows; placed after e1's
        # stage-1 so the PE arrives here well after the DVE router finishes.
        wT_r = sbr.tile([E, T], F32R, name="wT_r")
        for tt in range(TT):
            p = ps_r.tile([128, 512], F32, name=f"wtp{tt}", tag="ps_r")
            nc.tensor.transpose(p[0:E, 0:128], wt_tiles[tt][:], ident_f[:])
            nc.vector.tensor_copy(wT_r[:, tt * 128:(tt + 1) * 128].bitcast(F32R),
                                  p[0:E, 0:128].bitcast(F32R))
        wb_tiles = []
        for e in range(E_LOC):
            wbt = wb_pool.tile([128, T], F16, name=f"wb{e}")
            for th in range(TH):
                p = ps_r.tile([128, 512], F32, name=f"wbp{e}_{th}", tag="ps_r")
                nc.tensor.matmul(p[:], wselbc_sb[:, e * 128:(e + 1) * 128],
                                 wT_r[:, th * 512:(th + 1) * 512],
                                 start=True, stop=True)
                nc.vector.tensor_copy(wbt[:, th * 512:(th + 1) * 512], p[:])
            wb_tiles.append(wbt)

        # deferred routing-weight scale: in-place on the Pool engine.
        def scale_pass(ei):
            kind, e, ike = ENTRIES[ei]
            ab = a_tiles[a_base[ei]:a_base[ei] + ike]
            for th in range(TH):
                for it in range(ike):
                    sl = ab[it][:, th * 512:(th + 1) * 512]
                    nc.gpsimd.tensor_tensor(
                        sl, sl, wb_tiles[e][:, th * 512:(th + 1) * 512],
                        OP.mult)

        scale_pass(1)
        scale_pass(2)
        for ei in range(3, len(ENTRIES)):
            stage_a(ei)
            scale_pass(ei)

        # ---- stage 3: one 18-matmul fp16 PSUM chain per output tile -------
        if use_collective:
            bin_t = dram.tile([T, H], F32, name="rsin")
            target = bin_t
        else:
            target = out_d
        def b_chain(tt, hh, c0, cw, tag):
            op = ps_main.tile([128, 512], F32, name=f"o{tt}_{hh}_{c0}",
                              tag="ps")
            n = 0
            for ei, (kind, e, ike) in enumerate(ENTRIES):
                for it in range(ike):
                    nc.tensor.matmul(
                        op[:, 0:cw],
                        a_tiles[a_base[ei] + it][:, tt * 128:(tt + 1) * 128],
                        wd_sb[ei][:, it * H + hh * 512 + c0:
                                  it * H + hh * 512 + c0 + cw],
                        start=(n == 0), stop=(n == N_ITILES - 1))
                    n += 1
            st = stg_pool.tile([128, 512], F32, name=f"st{tt}_{hh}_{c0}",
                               tag=tag)
            nc.vector.tensor_scalar(st[:, 0:cw], op[:, 0:cw], CINV, None,
                                    OP.mult)
            nc.sync.dma_start(
                target[tt * 128:(tt + 1) * 128,
                       hh * 512 + c0:hh * 512 + c0 + cw],
                st[:, 0:cw])

        for tt in range(TT):
            for hh in range(NH):
                if tt == TT - 1 and hh == NH - 1:
                    # split the final tile so its evacuation+DMA pipeline
                    # overlaps the second half instead of the drain window
                    b_chain(tt, hh, 0, 256, "stg")
                    b_chain(tt, hh, 256, 256, "stg")
                else:
                    b_chain(tt, hh, 0, 512, "stg")

        # ---- ReduceScatter + output ---------------------------------------
        if use_collective:
            bout_t = dram.tile([out_rows, H], F32, name="rsout")
            nc.gpsimd.collective_compute(
                "ReduceScatter", OP.add,
                replica_groups=[list(range(num_devices))],
                ins=[bin_t.opt()], outs=[bout_t.opt()])
            nc.sync.dma_start(out_d[:], bout_t[:])
    nc.compile()
    return nc


_NC_CACHE = {}


def _get_module():
    key = "spmd"
    if key not in _NC_CACHE:
        _NC_CACHE[key] = build_module(use_collective=True, num_devices=N_CORES)
    return _NC_CACHE[key]


def _pack_rows(a, blk=128):
    """[R, C] -> [128, (R//128) * C]: row-tile r128 layout for one-DMA loads."""
    r, c = a.shape
    return np.ascontiguousarray(
        a.reshape(r // blk, blk, c).transpose(1, 0, 2).reshape(blk, -1))


def _fp8_split(a):
    """fp8e4 hi/lo split: a ~= hi + lo (both float8_e4m3)."""
    import ml_dtypes
    hi = a.astype(ml_dtypes.float8_e4m3)
    lo = (a - hi.astype(np.float32)).astype(ml_dtypes.float8_e4m3)
    return hi, lo


def _pack_w8(wT):
    """[H, C] f32 (pre-transposed weight) -> [128, (s, hp, j, C)] fp8 pair."""
    h, c = wT.shape
    hi, lo = _fp8_split(wT * SW)
    arr = np.stack([np.asarray(hi), np.asarray(lo)])        # [2, H, C]
    arr = arr.reshape(2, HP, 2, 128, c).transpose(3, 0, 1, 2, 4)
    return np.ascontiguousarray(arr.reshape(128, 2 * HP * 2 * c))


def _pack_x8(xT):
    """[H, T] f32 -> hi/lo [HP, 128, (j, T)] fp8 DoubleRow layout."""
    hi, lo = _fp8_split(xT * SX)
    out = []
    for a in (hi, lo):
        b = np.asarray(a).reshape(HP, 2, 128, T).transpose(0, 2, 1, 3)
        out.append(np.ascontiguousarray(b.reshape(HP, 128, 2 * T)))
    return out


def make_in_maps(hidden_states, gate_w, gate_bias, expert_gate, expert_up,
                 expert_down, shared_gate, shared_up, shared_down):
    x = np.asarray(hidden_states, np.float32).reshape(T, H)
    xt = np.ascontiguousarray(x.T)                       # [H, T]
    xh = xt.astype(np.float16)
    xl = (xt - xh.astype(np.float32)).astype(np.float16)
    x8h, x8l = _pack_x8(xt)
    gwt = np.ascontiguousarray(np.asarray(gate_w, np.float32).T)  # [H, E]
    gh = gwt.astype(np.float16)
    gl = (gwt - gh.astype(np.float32)).astype(np.float16)
    # pack gh/gl as [128, (ht, {gh,gl}, E)]
    ghl = np.concatenate(
        [gh.reshape(HK, 128, E)[:, :, None, :],
         gl.reshape(HK, 128, E)[:, :, None, :]], axis=2)  # [HK,128,2,E]
    ghl = np.ascontiguousarray(
        ghl.transpose(1, 0, 2, 3).reshape(128, HK * 2 * E))
    bias = np.broadcast_to(
        np.asarray(gate_bias, np.float32).reshape(1, E), (128, E))
    bias = np.ascontiguousarray(bias)
    eg = np.asarray(expert_gate, np.float32)
    eu = np.asarray(expert_up, np.float32)
    ed = np.asarray(expert_down, np.float32)
    sgT = np.asarray(shared_gate, np.float32).T          # [H, 2I]
    suT = np.asarray(shared_up, np.float32).T            # [H, 2I]
    sd = np.asarray(shared_down, np.float32)             # [H, 2I]
    in_maps = []
    for c in range(N_CORES):
        lo, hi = c * E_LOC, (c + 1) * E_LOC
        wselbc = np.zeros((E, E_LOC * 128), np.float32)
        for j in range(E_LOC):
            wselbc[lo + j, j * 128:(j + 1) * 128] = 2.5
        wg = np.stack([_pack_w8(eg[lo + j].T) for j in range(E_LOC)])
        wu = np.stack([_pack_w8(eu[lo + j].T) for j in range(E_LOC)])
        wd = np.stack([_pack_rows(ed[lo + j].T.astype(np.float16))
                       for j in range(E_LOC)])           # [E_LOC,128,IK*H]
        in_maps.append({
            "xh": xh, "xl": xl, "x8h": x8h, "x8l": x8l,
            "ghl": ghl, "bias": bias, "wselbc": wselbc,
            "wg": wg, "wu": wu, "wd": wd,
            "sg": _pack_w8(np.ascontiguousarray(sgT[:, c * ISH:(c + 1) * ISH])),
            "su": _pack_w8(np.ascontiguousarray(suT[:, c * ISH:(c + 1) * ISH])),
            "sd": _pack_rows(np.ascontiguousarray(
                sd[:, c * ISH:(c + 1) * ISH].T).astype(np.float16)),
        })
    return in_maps


def kernel(hidden_states, gate_w, gate_bias, expert_gate, expert_up,
           expert_down, shared_gate, shared_up, shared_down):
    import os
    # The axon NTFF trace hook is absent in this container; make sure the
    # PJRT execute path never tries to use it.
    os.environ.setdefault("BASS_NEVER_TRACE", "1")
    from concourse.bass_utils import run_bass_kernel_spmd
    nc = _get_module()
    in_maps = make_in_maps(hidden_states, gate_w, gate_bias, expert_gate,
                           expert_up, expert_down, shared_gate, shared_up,
                           shared_down)
    res = run_bass_kernel_spmd(nc, in_maps, core_ids=list(range(N_CORES)))
    out = np.concatenate([np.asarray(res.results[c]["out"], np.float32)
                          for c in range(N_CORES)], axis=0)
    return out.reshape(np.asarray(hidden_states).shape)
